# revision 45
# baseline (speedup 1.0000x reference)
"""Causal self-attention (RoPE) Trainium2 kernel, 8-way head-parallel.

Sharding: each of the 8 cores computes 2 of the 16 heads for all 4 batches
(tensor parallel over heads: W_qkv column-split, W_proj row-split). Host
pre-transposes x -> xT [C, B*T], slices per-core weights, and sum-reduces the
8 partial projection outputs (+ b_proj) — the standard row-parallel TP reduce.

Per-core dataflow (bf16 storage, bf16 matmuls, f32 PSUM):
  qT,kT = W.T @ xT   [feat, tok], bias added on PSUM evac, RoPE via
                     sign-baked cos/sin tables + rotate-half permutation
                     matmul on the PE
  v     = xT.T @ Wv  [tok, feat] directly (lhsT = x chunk), augmented with a
                     ones column per head -> v_aug [tok, (h, d|1)]
  S^T   = kT_jtile.T @ qT[i-window]   ragged on the causal diagonal
  P^T   = exp(S^T/8) (ACT, pair-tiles), triangular mask on diag 128x128 only
  y     = P^T.T @ v_aug accumulated over j tiles in PSUM ([i, d|den] layout,
                     K=128, M=128, N=65 -> 2.3x fewer PE rows than y^T form)
  y_norm= y * (1/den) per-partition (DVE), heads concatenated on free dim
  yT    via DMA XBAR transpose (off the PE), out = yT.T @ Wp per token tile
"""

import numpy as np

import concourse.bass as bass
import concourse.mybir as mybir
import concourse.tile as tile

F32 = mybir.dt.float32
BF16 = mybir.dt.bfloat16
AF = mybir.ActivationFunctionType
OP = mybir.AluOpType

# ---------------------------------------------------------------- tile patch
# This walrus build rejects >1 embedded sync-wait on sync-engine CTRL
# instructions; Tile's tail drain embeds one wait per outstanding semaphore.
# Split them across NOPs (1 wait each) before the drain.


def _patched_drain_and_barrier(self, tick_clock, wait_clock):
    from concourse.tile import ScopedClock

    nc = self.nc
    probe = nc.sync.nop(nofuse=True)
    wait_clock.add_sem_waits(probe.ins, ScopedClock({None: tick_clock.global_clock}))
    si = probe.ins.sync_info
    waits = list(si.on_wait) if si is not None and si.on_wait else []
    if len(waits) > 1:
        si.on_wait = waits[:1]
        for w in waits[1:]:
            nop = nc.sync.nop(nofuse=True)
            nsi = nop.ins.sync_info
            if nsi is None:
                nop.ins.sync_info = mybir.SyncInfo(on_wait=[w], on_update=[])
            else:
                nsi.on_wait = [w]
    nc.sync.drain()
    nc.all_engine_barrier()
    assert self.sems is not None
    popped = nc._tile_sem_poison_stack.pop()
    assert popped is self._sem_poison
    # chunk the sem clears: the range-encoded gpsimd drain (dma_reset) in this
    # walrus build rejects wide semaphore ranges ("ISA wrong length")
    sems = sorted(
        s.num if hasattr(s, "num") else s for s in self.sems.allocated().values()
    )
    for i in range(0, len(sems), 16):
        nc.clear_and_free_semaphores(sems[i : i + 16])
    nc.all_engine_barrier()


tile.TileContext._drain_and_barrier = _patched_drain_and_barrier


def _split_waits(nc):
    """Hoist all but one sync-wait per instruction onto same-engine NOPs
    (this walrus codegen supports a single embedded wait per instruction)."""
    n = 0
    for f in nc.m.functions:
        for blk in f.blocks:
            out = []
            changed = False
            for ins in blk.instructions:
                si = ins.sync_info
                if si is not None and si.on_wait and len(si.on_wait) > 1:
                    waits = list(si.on_wait)
                    for w in waits[:-1]:
                        n += 1
                        out.append(
                            mybir.InstNoOp(
                                name=f"wsplit{n}",
                                engine=ins.engine,
                                sync_info=mybir.SyncInfo(on_wait=[w], on_update=[]),
                                bass_nofuse=True,
                            )
                        )
                    si.on_wait = waits[-1:]
                    changed = True
                out.append(ins)
            if changed:
                blk.instructions = out
    return n


# ------------------------------------------------------------------- builder

HD = 64  # head dim (fixed)
ROPE_BASE = 10000.0
SCL = float(1.0 / np.sqrt(HD))


def build_nc(B, T, C, debug=False, split=True):
    """One core's program: 2 heads x B batches. T % 512 == 0, C % 128 == 0."""
    assert T % 512 == 0 and C % 128 == 0
    TOK = B * T
    KC = C // 128  # contraction chunks for QKV
    NCH = T // 512  # i-chunks per batch
    NJT = T // 128  # j-tiles per batch
    FW = 128  # qkv feature width per tensor (2 heads * 64)

    nc = bass.Bass()
    xT = nc.dram_tensor("xT", [C, TOK], BF16, kind="ExternalInput")
    wq = nc.dram_tensor("wq", [C, FW], BF16, kind="ExternalInput")
    wk = nc.dram_tensor("wk", [C, FW], BF16, kind="ExternalInput")
    wv = nc.dram_tensor("wv", [C, FW], BF16, kind="ExternalInput")
    bq = nc.dram_tensor("bq", [FW, 1], F32, kind="ExternalInput")
    bk = nc.dram_tensor("bk", [FW, 1], F32, kind="ExternalInput")
    bv = nc.dram_tensor("bv", [1, FW], F32, kind="ExternalInput")
    wp = nc.dram_tensor("wp", [FW, C], BF16, kind="ExternalInput")
    cosT = nc.dram_tensor("cosT", [FW, T], BF16, kind="ExternalInput")
    sinT = nc.dram_tensor("sinT", [FW, T], BF16, kind="ExternalInput")
    perm = nc.dram_tensor("perm", [128, 128], BF16, kind="ExternalInput")
    tri = nc.dram_tensor("tri", [128, 128], BF16, kind="ExternalInput")
    outp = nc.dram_tensor("outp", [TOK, C], BF16, kind="ExternalOutput")
    if debug:
        dbg_q = nc.dram_tensor("dbg_q", [FW, TOK], F32, kind="ExternalOutput")
        dbg_k = nc.dram_tensor("dbg_k", [FW, TOK], F32, kind="ExternalOutput")
        dbg_v = nc.dram_tensor("dbg_v", [B, 128, NJT * 2 * (HD + 1)], F32, kind="ExternalOutput")
        dbg_y = nc.dram_tensor("dbg_y", [B, 128, NJT * 128], F32, kind="ExternalOutput")
        dbg_pt = nc.dram_tensor("dbg_pt", [128, 1024], F32, kind="ExternalOutput")
        dbg_den = nc.dram_tensor("dbg_den", [128, 8], F32, kind="ExternalOutput")

    xT_r = xT[:, :].rearrange("(a p) t -> p a t", p=128)  # [128, KC, TOK]

    with tile.TileContext(nc) as tc:
        with (
            tc.tile_pool(name="const", bufs=1) as cpool,
            tc.tile_pool(name="xt", bufs=3) as xpool,
            tc.tile_pool(name="qk", bufs=2) as qkpool,
            tc.tile_pool(name="vv", bufs=2) as vpool,
            tc.tile_pool(name="rope", bufs=2) as rpool,
            tc.tile_pool(name="pt", bufs=6) as ptpool,
            tc.tile_pool(name="ysb", bufs=2) as ypool,
            tc.tile_pool(name="yt", bufs=16) as ytpool,
            tc.tile_pool(name="rcps", bufs=2) as rcpool,
            tc.tile_pool(name="outs", bufs=4) as opool,
            tc.tile_pool(name="ps_sp", bufs=2, space="PSUM") as ps_sp,
            tc.tile_pool(name="ps_y", bufs=1, space="PSUM") as ps_y,
            tc.tile_pool(name="ps_mm", bufs=2, space="PSUM") as ps_mm,
        ):
            # ---- constants ----
            # wq first, then the first x chunk (emitted by the caller right
            # after load_wq), then the rest — the SP DMA queue is in-order,
            # so this gets the first matmul started ASAP
            w_sb = {}

            def load_w(name, dram):
                t = cpool.tile([128, KC, FW], BF16, tag=name, name=name + "_sb")
                nc.sync.dma_start(
                    t[:, :, :], dram[:, :].rearrange("(a p) f -> p a f", p=128)
                )
                w_sb[name] = t

            load_w("wq", wq)
            b_sb = {}
            for name, dram in (("bq", bq), ("bk", bk)):
                t = cpool.tile([FW, 1], F32, tag=name, name=name + "_sb")
                nc.sync.dma_start(t[:, :], dram[:, :])
                b_sb[name] = t
            bv_b = cpool.tile([128, 2, HD], F32, tag="bv_b")
            nc.sync.dma_start(
                bv_b[:, :, :],
                bv[:, :].rearrange("o (h d) -> o h d", h=2).broadcast_to([128, 2, HD]),
            )
            perm_sb = cpool.tile([128, 128], BF16, tag="perm")
            nc.sync.dma_start(perm_sb[:, :], perm[:, :])
            tri_sb = cpool.tile([128, 128], BF16, tag="tri")
            nc.sync.dma_start(tri_sb[:, :], tri[:, :])
            cos_sb = cpool.tile([FW, T], BF16, tag="cos")
            sin_sb = cpool.tile([FW, T], BF16, tag="sin")
            wp2_sb = cpool.tile([FW, C], BF16, tag="wp2")

            def load_late_consts():
                load_w("wk", wk)
                load_w("wv", wv)
                nc.sync.dma_start(cos_sb[:, :], cosT[:, :])
                nc.sync.dma_start(sin_sb[:, :], sinT[:, :])
                nc.sync.dma_start(wp2_sb[:, :], wp[:, :])

            state = {}

            def alloc_qkv(b):
                st = state.setdefault(b, {})
                st["qT"] = qkpool.tile([FW, T], BF16, tag="qT", name=f"qT{b}")
                st["kT"] = qkpool.tile([FW, T], BF16, tag="kT", name=f"kT{b}")
                # v_aug[tok, jt, h, d|1]: per-head value tiles with a ones
                # column at d=HD (softmax denominator via the PV matmul)
                st["va"] = vpool.tile(
                    [128, NJT, 2, HD + 1], BF16, tag="va", name=f"va{b}"
                )
                nc.vector.memset(st["va"][:, :, :, HD], 1.0)

            def emit_qkv_piece(b, cn, which):
                """which in ('q', 'k', 'v'): one tensor's worth of a 512-token
                chunk. q/k include bias + RoPE; v is direct [tok, feat]."""
                st = state[b]
                tok0 = b * T
                ts0 = cn * 512
                if f"xt{cn}" not in st or st[f"xt{cn}"] is None:
                    # x chunk load serves q, k and v of this (b, cn)
                    xt = xpool.tile([128, KC, 512], BF16, tag="xt", name=f"xt{b}_{cn}")
                    st[f"xt{cn}"] = xt
                    nc.sync.dma_start(
                        xt[:, :, :], xT_r[:, :, tok0 + ts0 : tok0 + ts0 + 512]
                    )
                if which == "x":
                    return
                xt = st[f"xt{cn}"]
                if which in ("q", "k"):
                    wname = "wq" if which == "q" else "wk"
                    dest = st["qT"] if which == "q" else st["kT"]
                    ps = ps_mm.tile([128, 512], F32, tag="mm", name=f"qkps{b}_{cn}_{which}")
                    for kc in range(KC):
                        nc.tensor.matmul(
                            ps[:, :],
                            lhsT=w_sb[wname][:, kc, :],
                            rhs=xt[:, kc, :],
                            start=(kc == 0),
                            stop=(kc == KC - 1),
                        )
                    dch = dest[:, ts0 : ts0 + 512]
                    bias = b_sb["bq" if which == "q" else "bk"]
                    nc.vector.tensor_scalar_add(dch, ps[:, :], bias[:, :])
                    swp = ps_mm.tile([128, 512], F32, tag="mm", name=f"swp{b}_{cn}_{which}")
                    nc.tensor.matmul(
                        swp[:, :], lhsT=perm_sb[:, :], rhs=dch, start=True, stop=True
                    )
                    cc = cos_sb[:, ts0 : ts0 + 512]
                    ss = sin_sb[:, ts0 : ts0 + 512]
                    t1 = rpool.tile([128, 512], BF16, tag="t1", name=f"t1{b}_{cn}_{which}")
                    t2 = rpool.tile([128, 512], BF16, tag="t2", name=f"t2{b}_{cn}_{which}")
                    nc.gpsimd.tensor_tensor(t1[:, :], dch, cc, op=OP.mult)
                    nc.vector.tensor_tensor(t2[:, :], swp[:, :], ss, op=OP.mult)
                    nc.gpsimd.tensor_tensor(dch, t1[:, :], t2[:, :], op=OP.add)
                else:
                    # v directly in [tok, feat] layout: lhsT = x chunk
                    va = st["va"]
                    vps = ps_mm.tile([128, 4, 2, HD], F32, tag="mm", name=f"vps{b}_{cn}")
                    for s in range(4):
                        for kc in range(KC):
                            nc.tensor.matmul(
                                vps[:, s, :, :],
                                lhsT=xt[:, kc, s * 128 : s * 128 + 128],
                                rhs=w_sb["wv"][:, kc, :],
                                start=(kc == 0),
                                stop=(kc == KC - 1),
                            )
                    for s in range(4):
                        nc.vector.tensor_tensor(
                            va[:, cn * 4 + s, :, 0:HD],
                            vps[:, s, :, :],
                            bv_b[:, :, :],
                            op=OP.add,
                        )

            # ---- attention ----
            # Per (b, ic): j-tiles jt = 0..4(ic+1)-1, each S^T [128j, N] with
            # N ragged on the diagonal (i-window = [(ic+1)*512-N, (ic+1)*512)).
            # jts are processed in GROUPS of 2 (jtA, jtB): per head one
            # [128,1024] psum tile holds jtA at [0:N_A] and jtB at
            # [512:512+N_B]. The h0/h1 S matmuls for the same jt are emitted
            # adjacently: K=64 at partitions 0/64 -> disjoint PE row-groups,
            # so they run concurrently.
            def emit_s_group(b, ic, g, tag):
                st = state[b]
                qT, kT = st["qT"], st["kT"]
                jts = [2 * g, 2 * g + 1]
                Ns, i_los = [], []
                for jt in jts:
                    r = jt - 4 * ic
                    N = 512 if r < 0 else 512 - 128 * r
                    Ns.append(N)
                    i_los.append((ic + 1) * 512 - N)
                sps = {
                    h: ps_sp.tile([128, 1024], F32, tag="sp", name=f"sps{tag}_{h}")
                    for h in range(2)
                }
                for li, jt in enumerate(jts):
                    N, i_lo = Ns[li], i_los[li]
                    for h in range(2):
                        hr0 = h * HD
                        nc.tensor.matmul(
                            sps[h][:, li * 512 : li * 512 + N],
                            lhsT=kT[hr0 : hr0 + HD, jt * 128 : jt * 128 + 128],
                            rhs=qT[hr0 : hr0 + HD, i_lo : i_lo + N],
                            start=True,
                            stop=True,
                        )
                pt = {}
                for h in range(2):
                    pt[h] = ptpool.tile([128, 1024], BF16, tag="pt", name=f"pt{tag}_{h}")
                    if Ns[0] == 512:  # contiguous [0 : 512+N_B]
                        nc.scalar.activation(
                            pt[h][:, 0 : 512 + Ns[1]],
                            sps[h][:, 0 : 512 + Ns[1]],
                            AF.Exp,
                            scale=SCL,
                        )
                    else:
                        for li in range(2):
                            nc.scalar.activation(
                                pt[h][:, li * 512 : li * 512 + Ns[li]],
                                sps[h][:, li * 512 : li * 512 + Ns[li]],
                                AF.Exp,
                                scale=SCL,
                            )
                for li, jt in enumerate(jts):
                    if jt >= 4 * ic:  # triangular mask on the it==jt block
                        for h in range(2):
                            nc.gpsimd.tensor_tensor(
                                pt[h][:, li * 512 : li * 512 + 128],
                                pt[h][:, li * 512 : li * 512 + 128],
                                tri_sb[:, :],
                                op=OP.mult,
                            )
                if debug and b == 0 and ic == 0 and g == 0:
                    nc.gpsimd.dma_start(dbg_pt[:, :], pt[0][:, :])
                return (jts, Ns, i_los, pt)

            def emit_pv(b, ic, grp, yps):
                jts, Ns, i_los, pt = grp
                va = state[b]["va"]
                for li, jt in enumerate(jts):
                    N, i_lo = Ns[li], i_los[li]
                    for h in range(2):
                        for it in range(i_lo // 128, 4 * ic + 4):
                            col = li * 512 + (it * 128 - i_lo)
                            # start=True resets the has-written bits for the
                            # WHOLE psum bank: only the very first matmul into
                            # this bank may use it; later regions' first
                            # writes land on cleared bits and overwrite.
                            nc.tensor.matmul(
                                yps[h][:, it - 4 * ic, 0 : HD + 1],
                                lhsT=pt[h][:, col : col + 128],
                                rhs=va[:, jt, h, :],
                                start=(jt == 0 and it == 4 * ic),
                                stop=(jt == it),
                                skip_group_check=True,
                            )

            def emit_attn_ic(b, ic, pop_filler):
                """pop_filler(): emits one queued filler piece (or nothing).
                Called once per jt-group slot to densify the PE stream."""
                st = state[b]
                yps = {}
                for h in range(2):
                    # padded to 128 so each accumulation region is 512B-aligned
                    yps[h] = ps_y.tile(
                        [128, 4, 128], F32, tag=f"y{h}", name=f"yps{b}_{ic}_{h}"
                    )
                ng = 2 * (ic + 1)
                pend = []  # PV deferred 1 group (2 jts) behind S
                for g in range(ng):
                    cur = emit_s_group(b, ic, g, f"{b}_{ic}_{g}")
                    pop_filler()
                    if len(pend) >= 1:
                        emit_pv(b, ic, pend.pop(0), yps)
                    pend.append(cur)
                for p in pend:
                    emit_pv(b, ic, p, yps)
                # normalization: y_sb[tok, it, h, d] = yps * 1/den
                y_sb = ypool.tile([128, 4, 2, HD], BF16, tag="ysb", name=f"ysb{b}_{ic}")
                st[f"ysb{ic}"] = y_sb
                rcp = rcpool.tile([128, 2, 4], F32, tag="rcp", name=f"rcp{b}_{ic}")
                for h in range(2):
                    nc.vector.reciprocal(rcp[:, h, :], yps[h][:, :, HD])
                    nc.vector.tensor_tensor(
                        y_sb[:, :, h, :],
                        yps[h][:, :, 0:HD],
                        rcp[:, h, :].broadcast_to([128, 4, HD]),
                        op=OP.mult,
                    )
                if debug and b == 0 and ic == 0:
                    dent = rcpool.tile([128, 8], F32, tag="dent", name="dent")
                    for h in range(2):
                        nc.vector.tensor_copy(
                            dent[:, h * 4 : h * 4 + 4], yps[h][:, :, HD]
                        )
                    nc.gpsimd.dma_start(dbg_den[:, :], dent[:, :])
                # transpose each [128 tok, 128 feat] i-tile via DMA XBAR
                st[f"yT{ic}"] = []
                for it in range(4):
                    yt = ytpool.tile([128, 128], BF16, tag="yt", name=f"yt{b}_{ic}_{it}")
                    nc.sync.dma_start(yt[:, :], y_sb[:, it, :, :], transpose=True)
                    st[f"yT{ic}"].append(yt)
                if debug:
                    nc.gpsimd.dma_start(
                        dbg_y[b, :, ic * 512 : ic * 512 + 512],
                        y_sb[:, :, :, :].rearrange("p a h d -> p (a h d)"),
                    )

            def emit_proj_piece(b, ic, it):
                st = state[b]
                yt = st[f"yT{ic}"][it]
                tr0 = b * T + ic * 512 + it * 128
                ot = opool.tile([128, C], BF16, tag="ot", name=f"ot{b}_{ic}_{it}")
                for fc in range(2):
                    pp = ps_mm.tile([128, 512], F32, tag="mm", name=f"pp{b}_{ic}_{it}_{fc}")
                    nc.tensor.matmul(
                        pp[:, :],
                        lhsT=yt[:, :],
                        rhs=wp2_sb[:, fc * 512 : fc * 512 + 512],
                        start=True,
                        stop=True,
                    )
                    nc.vector.tensor_copy(ot[:, fc * 512 : fc * 512 + 512], pp[:, :])
                nc.sync.dma_start(outp[tr0 : tr0 + 128, :], ot[:, :])

            # ---- emission: software-pipelined; qkv runs one chunk ahead of
            # the attention that consumes it. Filler pieces (qkv, proj) are
            # banked in a deadline queue and dispensed one per jt-group slot
            # so attention-heavy chunks keep a dense PE stream. ----
            work_q = []  # list of (deadline_linear_idx, fn), kept sorted

            def push_work(deadline, fn):
                work_q.append((deadline, fn))

            def flush_due(now):
                work_q.sort(key=lambda e: e[0])
                while work_q and work_q[0][0] <= now:
                    work_q.pop(0)[1]()

            slots_left = [2 * sum(ic + 1 for ic in range(NCH)) * B]  # 80

            def pop_one():
                # pop harder when the backlog outpaces the remaining slots
                n = max(1, (len(work_q) + slots_left[0] - 1) // max(slots_left[0], 1))
                for _ in range(n):
                    if work_q:
                        work_q.pop(0)[1]()
                slots_left[0] -= 1

            alloc_qkv(0)
            emit_qkv_piece(0, 0, "x")
            load_late_consts()
            for which in ("q", "k", "v"):
                emit_qkv_piece(0, 0, which)
            for b in range(B):
                for ic in range(NCH):
                    now = b * NCH + ic
                    # next qkv chunk: (b, ic+1), rolling into (b+1, 0);
                    # must be emitted before attention (b, ic+1) -> deadline
                    nb, ncn = (b, ic + 1) if ic + 1 < NCH else (b + 1, 0)
                    if nb < B:
                        if ncn == 0:
                            alloc_qkv(nb)
                        for which in ("q", "k", "v"):
                            push_work(
                                nb * NCH + ncn,
                                lambda b_=nb, cn_=ncn, w_=which: emit_qkv_piece(b_, cn_, w_),
                            )
                    pb, pic = (b, ic - 1) if ic > 0 else (b - 1, NCH - 1)
                    if pb >= 0:
                        for it in range(4):
                            push_work(
                                now + 3,
                                lambda pb_=pb, pic_=pic, it_=it: emit_proj_piece(pb_, pic_, it_),
                            )
                    flush_due(now)
                    emit_attn_ic(b, ic, pop_one)
                if debug:
                    st = state[b]
                    nc.gpsimd.dma_start(dbg_q[:, b * T : b * T + T], st["qT"][:, :])
                    nc.gpsimd.dma_start(dbg_k[:, b * T : b * T + T], st["kT"][:, :])
                    nc.gpsimd.dma_start(
                        dbg_v[b, :, :],
                        st["va"][:, :, :, :].rearrange("p a h d -> p (a h d)"),
                    )
            while work_q:
                pop_one()
            for it in range(4):
                emit_proj_piece(B - 1, NCH - 1, it)
    if split:
        _split_waits(nc)
    return nc


# ---------------------------------------------------------------- host side


def make_tables(T):
    inv_freq = 1.0 / (ROPE_BASE ** (np.arange(0, HD, 2, dtype=np.float32) / HD))
    pos = np.arange(T, dtype=np.float32)
    freqs = pos[:, None] * inv_freq[None, :]  # [T, 32]
    cos = np.cos(freqs).astype(np.float32)  # [T, 32] (same for both halves)
    sin = np.sin(freqs).astype(np.float32)
    cosT64 = np.concatenate([cos.T, cos.T], axis=0)  # [64, T]
    sinT64 = np.concatenate([-sin.T, sin.T], axis=0)  # sign-baked rotate_half
    cosT = np.concatenate([cosT64, cosT64], axis=0).copy()  # [128, T] two heads
    sinT = np.concatenate([sinT64, sinT64], axis=0).copy()
    return cosT, sinT


def make_perm():
    # perm[k, m] = 1 iff m == (k+32) % 64 within each 64-row head block
    p = np.zeros((128, 128), dtype=np.float32)
    for hb in range(2):
        for k in range(HD):
            p[hb * HD + k, hb * HD + (k + 32) % HD] = 1.0
    return p


def make_tri():
    # tri[p, f] = 1.0 if p <= f (keep j <= i within a diagonal 128x128 tile)
    p = np.arange(128)[:, None]
    f = np.arange(128)[None, :]
    return (p <= f).astype(np.float32)


def make_in_maps(x, W_qkv, b_qkv, W_proj, n_cores):
    B, T, C = x.shape
    import ml_dtypes

    xT = np.ascontiguousarray(x.reshape(B * T, C).T.astype(ml_dtypes.bfloat16))
    cosT, sinT = make_tables(T)
    tri = make_tri()
    perm = make_perm()
    in_maps = []
    for c in range(n_cores):
        h0 = 2 * c * HD  # first head's column offset (2 heads per core)
        sl = slice(h0, h0 + 128)
        in_maps.append(
            {
                "xT": xT,
                "wq": np.ascontiguousarray(W_qkv[:, sl].astype(ml_dtypes.bfloat16)),
                "wk": np.ascontiguousarray(
                    W_qkv[:, C:][:, sl].astype(ml_dtypes.bfloat16)
                ),
                "wv": np.ascontiguousarray(
                    W_qkv[:, 2 * C :][:, sl].astype(ml_dtypes.bfloat16)
                ),
                "bq": np.ascontiguousarray(b_qkv[sl].reshape(128, 1)),
                "bk": np.ascontiguousarray(b_qkv[C:][sl].reshape(128, 1)),
                "bv": np.ascontiguousarray(b_qkv[2 * C :][sl].reshape(1, 128)),
                "wp": np.ascontiguousarray(W_proj[sl, :].astype(ml_dtypes.bfloat16)),
                "cosT": cosT.astype(ml_dtypes.bfloat16),
                "sinT": sinT.astype(ml_dtypes.bfloat16),
                "perm": perm.astype(ml_dtypes.bfloat16),
                "tri": tri.astype(ml_dtypes.bfloat16),
            }
        )
    return in_maps


_NC_CACHE = {}


def _get_nc(B, T, C):
    key = (B, T, C)
    if key not in _NC_CACHE:
        _NC_CACHE[key] = build_nc(B, T, C)
    return _NC_CACHE[key]


def kernel(x, W_qkv, b_qkv, W_proj, b_proj):
    from concourse.bass_utils import run_bass_kernel_spmd

    x = np.asarray(x, dtype=np.float32)
    W_qkv = np.asarray(W_qkv, dtype=np.float32)
    b_qkv = np.asarray(b_qkv, dtype=np.float32)
    W_proj = np.asarray(W_proj, dtype=np.float32)
    b_proj = np.asarray(b_proj, dtype=np.float32)
    B, T, C = x.shape
    n_cores = 8
    nc = _get_nc(B, T, C)
    in_maps = make_in_maps(x, W_qkv, b_qkv, W_proj, n_cores)
    res = run_bass_kernel_spmd(nc, in_maps, core_ids=list(range(n_cores)))
    out = np.zeros((B * T, C), dtype=np.float32)
    for r in res.results:
        out += r["outp"].astype(np.float32)
    out += b_proj[None, :]
    return out.reshape(B, T, C)


# revision 46
# speedup vs baseline: 1.1464x; 1.1464x over previous
"""Causal self-attention (RoPE) Trainium2 kernel, 8-way head-parallel.

Sharding: each of the 8 cores computes 2 of the 16 heads for all 4 batches
(tensor parallel over heads: W_qkv column-split, W_proj row-split). Host
pre-transposes x -> xT [C, B*T], slices per-core weights, and sum-reduces the
8 partial projection outputs (+ b_proj) — the standard row-parallel TP reduce.

Per-core dataflow (bf16 storage, bf16 matmuls, f32 PSUM):
  qT,kT = W.T @ xT   [feat, tok], bias added on PSUM evac, RoPE via
                     sign-baked cos/sin tables + rotate-half permutation
                     matmul on the PE
  v     = xT.T @ Wv  [tok, feat] directly (lhsT = x chunk), augmented with a
                     ones column per head -> v_aug [tok, (h, d|1)]
  S^T   = kT_jtile.T @ qT[i-window]   ragged on the causal diagonal
  P^T   = exp(S^T/8) (ACT, pair-tiles), triangular mask on diag 128x128 only
  y     = P^T.T @ v_aug accumulated over j tiles in PSUM ([i, d|den] layout,
                     K=128, M=128, N=65 -> 2.3x fewer PE rows than y^T form)
  y_norm= y * (1/den) per-partition (DVE), heads concatenated on free dim
  yT    via DMA XBAR transpose (off the PE), out = yT.T @ Wp per token tile
"""

import numpy as np

import concourse.bass as bass
import concourse.mybir as mybir
import concourse.tile as tile

F32 = mybir.dt.float32
BF16 = mybir.dt.bfloat16
AF = mybir.ActivationFunctionType
OP = mybir.AluOpType

# ---------------------------------------------------------------- tile patch
# This walrus build rejects >1 embedded sync-wait on sync-engine CTRL
# instructions; Tile's tail drain embeds one wait per outstanding semaphore.
# Split them across NOPs (1 wait each) before the drain.


def _patched_drain_and_barrier(self, tick_clock, wait_clock):
    from concourse.tile import ScopedClock

    nc = self.nc
    probe = nc.sync.nop(nofuse=True)
    wait_clock.add_sem_waits(probe.ins, ScopedClock({None: tick_clock.global_clock}))
    si = probe.ins.sync_info
    waits = list(si.on_wait) if si is not None and si.on_wait else []
    if len(waits) > 1:
        si.on_wait = waits[:1]
        for w in waits[1:]:
            nop = nc.sync.nop(nofuse=True)
            nsi = nop.ins.sync_info
            if nsi is None:
                nop.ins.sync_info = mybir.SyncInfo(on_wait=[w], on_update=[])
            else:
                nsi.on_wait = [w]
    nc.sync.drain()
    nc.all_engine_barrier()
    assert self.sems is not None
    popped = nc._tile_sem_poison_stack.pop()
    assert popped is self._sem_poison
    # chunk the sem clears: the range-encoded gpsimd drain (dma_reset) in this
    # walrus build rejects wide semaphore ranges ("ISA wrong length")
    sems = sorted(
        s.num if hasattr(s, "num") else s for s in self.sems.allocated().values()
    )
    for i in range(0, len(sems), 16):
        nc.clear_and_free_semaphores(sems[i : i + 16])
    nc.all_engine_barrier()


tile.TileContext._drain_and_barrier = _patched_drain_and_barrier


def _split_waits(nc):
    """Hoist all but one sync-wait per instruction onto same-engine NOPs
    (this walrus codegen supports a single embedded wait per instruction)."""
    n = 0
    for f in nc.m.functions:
        for blk in f.blocks:
            out = []
            changed = False
            for ins in blk.instructions:
                si = ins.sync_info
                if si is not None and si.on_wait and len(si.on_wait) > 1:
                    waits = list(si.on_wait)
                    for w in waits[:-1]:
                        n += 1
                        out.append(
                            mybir.InstNoOp(
                                name=f"wsplit{n}",
                                engine=ins.engine,
                                sync_info=mybir.SyncInfo(on_wait=[w], on_update=[]),
                                bass_nofuse=True,
                            )
                        )
                    si.on_wait = waits[-1:]
                    changed = True
                out.append(ins)
            if changed:
                blk.instructions = out
    return n


# ------------------------------------------------------------------- builder

HD = 64  # head dim (fixed)
ROPE_BASE = 10000.0
SCL = float(1.0 / np.sqrt(HD))


def build_nc(B, T, C, debug=False, split=True):
    """One core's program: 2 heads x B batches. T % 512 == 0, C % 128 == 0."""
    assert T % 512 == 0 and C % 128 == 0
    TOK = B * T
    KC = C // 128  # contraction chunks for QKV
    NCH = T // 512  # i-chunks per batch
    NJT = T // 128  # j-tiles per batch
    FW = 128  # qkv feature width per tensor (2 heads * 64)

    nc = bass.Bass()
    xT = nc.dram_tensor("xT", [C, TOK], BF16, kind="ExternalInput")
    wq = nc.dram_tensor("wq", [C, FW], BF16, kind="ExternalInput")
    wk = nc.dram_tensor("wk", [C, FW], BF16, kind="ExternalInput")
    wv = nc.dram_tensor("wv", [C, FW], BF16, kind="ExternalInput")
    bq = nc.dram_tensor("bq", [FW, 1], F32, kind="ExternalInput")
    bk = nc.dram_tensor("bk", [FW, 1], F32, kind="ExternalInput")
    bv = nc.dram_tensor("bv", [1, FW], F32, kind="ExternalInput")
    wp = nc.dram_tensor("wp", [FW, C], BF16, kind="ExternalInput")
    cosT = nc.dram_tensor("cosT", [FW, T], BF16, kind="ExternalInput")
    sinT = nc.dram_tensor("sinT", [FW, T], BF16, kind="ExternalInput")
    perm = nc.dram_tensor("perm", [128, 128], BF16, kind="ExternalInput")
    tri = nc.dram_tensor("tri", [128, 128], BF16, kind="ExternalInput")
    outp = nc.dram_tensor("outp", [TOK, C], BF16, kind="ExternalOutput")
    if debug:
        dbg_q = nc.dram_tensor("dbg_q", [FW, TOK], F32, kind="ExternalOutput")
        dbg_k = nc.dram_tensor("dbg_k", [FW, TOK], F32, kind="ExternalOutput")
        dbg_v = nc.dram_tensor("dbg_v", [B, 128, NJT * 2 * (HD + 1)], F32, kind="ExternalOutput")
        dbg_y = nc.dram_tensor("dbg_y", [B, 128, NJT * 128], F32, kind="ExternalOutput")
        dbg_pt = nc.dram_tensor("dbg_pt", [128, 1024], F32, kind="ExternalOutput")
        dbg_den = nc.dram_tensor("dbg_den", [128, 8], F32, kind="ExternalOutput")

    xT_r = xT[:, :].rearrange("(a p) t -> p a t", p=128)  # [128, KC, TOK]

    with tile.TileContext(nc) as tc:
        with (
            tc.tile_pool(name="const", bufs=1) as cpool,
            tc.tile_pool(name="xt", bufs=3) as xpool,
            tc.tile_pool(name="qk", bufs=2) as qkpool,
            tc.tile_pool(name="vv", bufs=2) as vpool,
            tc.tile_pool(name="rope", bufs=2) as rpool,
            tc.tile_pool(name="pt", bufs=6) as ptpool,
            tc.tile_pool(name="ysb", bufs=2) as ypool,
            tc.tile_pool(name="yt", bufs=16) as ytpool,
            tc.tile_pool(name="rcps", bufs=2) as rcpool,
            tc.tile_pool(name="outs", bufs=4) as opool,
            tc.tile_pool(name="ps_sp", bufs=2, space="PSUM") as ps_sp,
            tc.tile_pool(name="ps_y", bufs=1, space="PSUM") as ps_y,
            tc.tile_pool(name="ps_mm", bufs=2, space="PSUM") as ps_mm,
        ):
            # ---- constants ----
            # wq first, then the first x chunk (emitted by the caller right
            # after load_wq), then the rest — the SP DMA queue is in-order,
            # so this gets the first matmul started ASAP
            w_sb = {}

            def load_w(name, dram):
                t = cpool.tile([128, KC, FW], BF16, tag=name, name=name + "_sb")
                nc.sync.dma_start(
                    t[:, :, :], dram[:, :].rearrange("(a p) f -> p a f", p=128)
                )
                w_sb[name] = t

            load_w("wq", wq)
            b_sb = {}
            for name, dram in (("bq", bq), ("bk", bk)):
                t = cpool.tile([FW, 1], F32, tag=name, name=name + "_sb")
                nc.sync.dma_start(t[:, :], dram[:, :])
                b_sb[name] = t
            bv_b = cpool.tile([128, 2, HD], F32, tag="bv_b")
            nc.sync.dma_start(
                bv_b[:, :, :],
                bv[:, :].rearrange("o (h d) -> o h d", h=2).broadcast_to([128, 2, HD]),
            )
            perm_sb = cpool.tile([128, 128], BF16, tag="perm")
            nc.sync.dma_start(perm_sb[:, :], perm[:, :])
            tri_sb = cpool.tile([128, 128], BF16, tag="tri")
            nc.sync.dma_start(tri_sb[:, :], tri[:, :])
            cos_sb = cpool.tile([FW, T], BF16, tag="cos")
            sin_sb = cpool.tile([FW, T], BF16, tag="sin")
            wp2_sb = cpool.tile([FW, C], BF16, tag="wp2")

            def load_late_consts():
                load_w("wk", wk)
                load_w("wv", wv)
                nc.sync.dma_start(cos_sb[:, :], cosT[:, :])
                nc.sync.dma_start(sin_sb[:, :], sinT[:, :])
                nc.sync.dma_start(wp2_sb[:, :], wp[:, :])

            state = {}

            def alloc_qkv(b):
                st = state.setdefault(b, {})
                st["qT"] = qkpool.tile([FW, T], BF16, tag="qT", name=f"qT{b}")
                st["kT"] = qkpool.tile([FW, T], BF16, tag="kT", name=f"kT{b}")
                # v_aug[tok, jt, h, d|1]: per-head value tiles with a ones
                # column at d=HD (softmax denominator via the PV matmul)
                st["va"] = vpool.tile(
                    [128, NJT, 2, HD + 1], BF16, tag="va", name=f"va{b}"
                )
                nc.vector.memset(st["va"][:, :, :, HD], 1.0)

            def emit_qkv_piece(b, cn, which):
                """which in ('q', 'k', 'v'): one tensor's worth of a 512-token
                chunk. q/k include bias + RoPE; v is direct [tok, feat]."""
                st = state[b]
                tok0 = b * T
                ts0 = cn * 512
                if f"xt{cn}" not in st or st[f"xt{cn}"] is None:
                    # x chunk load serves q, k and v of this (b, cn)
                    xt = xpool.tile([128, KC, 512], BF16, tag="xt", name=f"xt{b}_{cn}")
                    st[f"xt{cn}"] = xt
                    nc.sync.dma_start(
                        xt[:, :, :], xT_r[:, :, tok0 + ts0 : tok0 + ts0 + 512]
                    )
                if which == "x":
                    return
                xt = st[f"xt{cn}"]
                if which in ("q", "k"):
                    wname = "wq" if which == "q" else "wk"
                    dest = st["qT"] if which == "q" else st["kT"]
                    ps = ps_mm.tile([128, 512], F32, tag="mm", name=f"qkps{b}_{cn}_{which}")
                    for kc in range(KC):
                        nc.tensor.matmul(
                            ps[:, :],
                            lhsT=w_sb[wname][:, kc, :],
                            rhs=xt[:, kc, :],
                            start=(kc == 0),
                            stop=(kc == KC - 1),
                        )
                    dch = dest[:, ts0 : ts0 + 512]
                    bias = b_sb["bq" if which == "q" else "bk"]
                    nc.vector.tensor_scalar_add(dch, ps[:, :], bias[:, :])
                    swp = ps_mm.tile([128, 512], F32, tag="mm", name=f"swp{b}_{cn}_{which}")
                    nc.tensor.matmul(
                        swp[:, :], lhsT=perm_sb[:, :], rhs=dch, start=True, stop=True
                    )
                    cc = cos_sb[:, ts0 : ts0 + 512]
                    ss = sin_sb[:, ts0 : ts0 + 512]
                    t1 = rpool.tile([128, 512], BF16, tag="t1", name=f"t1{b}_{cn}_{which}")
                    t2 = rpool.tile([128, 512], BF16, tag="t2", name=f"t2{b}_{cn}_{which}")
                    nc.gpsimd.tensor_tensor(t1[:, :], dch, cc, op=OP.mult)
                    nc.vector.tensor_tensor(t2[:, :], swp[:, :], ss, op=OP.mult)
                    nc.gpsimd.tensor_tensor(dch, t1[:, :], t2[:, :], op=OP.add)
                else:
                    # v directly in [tok, feat] layout: lhsT = x chunk
                    va = st["va"]
                    vps = ps_mm.tile([128, 4, 2, HD], F32, tag="mm", name=f"vps{b}_{cn}")
                    for s in range(4):
                        for kc in range(KC):
                            nc.tensor.matmul(
                                vps[:, s, :, :],
                                lhsT=xt[:, kc, s * 128 : s * 128 + 128],
                                rhs=w_sb["wv"][:, kc, :],
                                start=(kc == 0),
                                stop=(kc == KC - 1),
                            )
                    for s in range(4):
                        nc.vector.tensor_tensor(
                            va[:, cn * 4 + s, :, 0:HD],
                            vps[:, s, :, :],
                            bv_b[:, :, :],
                            op=OP.add,
                        )

            # ---- attention ----
            # Per (b, ic): j-tiles jt = 0..4(ic+1)-1, each S^T [128j, N] with
            # N ragged on the diagonal (i-window = [(ic+1)*512-N, (ic+1)*512)).
            # jts are processed in GROUPS of 2 (jtA, jtB): per head one
            # [128,1024] psum tile holds jtA at [0:N_A] and jtB at
            # [512:512+N_B]. The h0/h1 S matmuls for the same jt are emitted
            # adjacently: K=64 at partitions 0/64 -> disjoint PE row-groups,
            # so they run concurrently.
            def emit_s_group(b, ic, g, tag):
                st = state[b]
                qT, kT = st["qT"], st["kT"]
                jts = [2 * g, 2 * g + 1]
                Ns, i_los = [], []
                for jt in jts:
                    r = jt - 4 * ic
                    N = 512 if r < 0 else 512 - 128 * r
                    Ns.append(N)
                    i_los.append((ic + 1) * 512 - N)
                sps = {
                    h: ps_sp.tile([128, 1024], F32, tag="sp", name=f"sps{tag}_{h}")
                    for h in range(2)
                }
                for li, jt in enumerate(jts):
                    N, i_lo = Ns[li], i_los[li]
                    for h in range(2):
                        hr0 = h * HD
                        nc.tensor.matmul(
                            sps[h][:, li * 512 : li * 512 + N],
                            lhsT=kT[hr0 : hr0 + HD, jt * 128 : jt * 128 + 128],
                            rhs=qT[hr0 : hr0 + HD, i_lo : i_lo + N],
                            start=True,
                            stop=True,
                        )
                pt = {}
                for h in range(2):
                    pt[h] = ptpool.tile([128, 1024], BF16, tag="pt", name=f"pt{tag}_{h}")
                    if Ns[0] == 512:  # contiguous [0 : 512+N_B]
                        nc.scalar.activation(
                            pt[h][:, 0 : 512 + Ns[1]],
                            sps[h][:, 0 : 512 + Ns[1]],
                            AF.Exp,
                            scale=SCL,
                        )
                    else:
                        for li in range(2):
                            nc.scalar.activation(
                                pt[h][:, li * 512 : li * 512 + Ns[li]],
                                sps[h][:, li * 512 : li * 512 + Ns[li]],
                                AF.Exp,
                                scale=SCL,
                            )
                for li, jt in enumerate(jts):
                    if jt >= 4 * ic:  # triangular mask on the it==jt block
                        for h in range(2):
                            nc.gpsimd.tensor_tensor(
                                pt[h][:, li * 512 : li * 512 + 128],
                                pt[h][:, li * 512 : li * 512 + 128],
                                tri_sb[:, :],
                                op=OP.mult,
                            )
                if debug and b == 0 and ic == 0 and g == 0:
                    nc.gpsimd.dma_start(dbg_pt[:, :], pt[0][:, :])
                return (jts, Ns, i_los, pt)

            def emit_pv(b, ic, grp, yps):
                jts, Ns, i_los, pt = grp
                va = state[b]["va"]
                for li, jt in enumerate(jts):
                    N, i_lo = Ns[li], i_los[li]
                    for h in range(2):
                        for it in range(i_lo // 128, 4 * ic + 4):
                            col = li * 512 + (it * 128 - i_lo)
                            # start=True resets the has-written bits for the
                            # WHOLE psum bank: only the very first matmul into
                            # this bank may use it; later regions' first
                            # writes land on cleared bits and overwrite.
                            nc.tensor.matmul(
                                yps[h][:, it - 4 * ic, 0 : HD + 1],
                                lhsT=pt[h][:, col : col + 128],
                                rhs=va[:, jt, h, :],
                                start=(jt == 0 and it == 4 * ic),
                                stop=(jt == it),
                                skip_group_check=True,
                            )

            def emit_attn_ic(b, ic, pop_filler):
                """pop_filler(): emits one queued filler piece (or nothing).
                Called once per jt-group slot to densify the PE stream."""
                st = state[b]
                yps = {}
                for h in range(2):
                    # padded to 128 so each accumulation region is 512B-aligned
                    yps[h] = ps_y.tile(
                        [128, 4, 128], F32, tag=f"y{h}", name=f"yps{b}_{ic}_{h}"
                    )
                ng = 2 * (ic + 1)
                pend = []  # PV deferred 1 group (2 jts) behind S
                for g in range(ng):
                    cur = emit_s_group(b, ic, g, f"{b}_{ic}_{g}")
                    pop_filler()
                    if len(pend) >= 1:
                        emit_pv(b, ic, pend.pop(0), yps)
                    pend.append(cur)
                for p in pend:
                    emit_pv(b, ic, p, yps)
                # normalization: y_sb[tok, it, h, d] = yps * 1/den
                y_sb = ypool.tile([128, 4, 2, HD], BF16, tag="ysb", name=f"ysb{b}_{ic}")
                st[f"ysb{ic}"] = y_sb
                rcp = rcpool.tile([128, 2, 4], F32, tag="rcp", name=f"rcp{b}_{ic}")
                for h in range(2):
                    nc.vector.reciprocal(rcp[:, h, :], yps[h][:, :, HD])
                    nc.vector.tensor_tensor(
                        y_sb[:, :, h, :],
                        yps[h][:, :, 0:HD],
                        rcp[:, h, :].broadcast_to([128, 4, HD]),
                        op=OP.mult,
                    )
                if debug and b == 0 and ic == 0:
                    dent = rcpool.tile([128, 8], F32, tag="dent", name="dent")
                    for h in range(2):
                        nc.vector.tensor_copy(
                            dent[:, h * 4 : h * 4 + 4], yps[h][:, :, HD]
                        )
                    nc.gpsimd.dma_start(dbg_den[:, :], dent[:, :])
                # transpose each [128 tok, 128 feat] i-tile via DMA XBAR
                st[f"yT{ic}"] = []
                for it in range(4):
                    yt = ytpool.tile([128, 128], BF16, tag="yt", name=f"yt{b}_{ic}_{it}")
                    nc.sync.dma_start(yt[:, :], y_sb[:, it, :, :], transpose=True)
                    st[f"yT{ic}"].append(yt)
                if debug:
                    nc.gpsimd.dma_start(
                        dbg_y[b, :, ic * 512 : ic * 512 + 512],
                        y_sb[:, :, :, :].rearrange("p a h d -> p (a h d)"),
                    )

            def emit_proj_piece(b, ic, it):
                st = state[b]
                yt = st[f"yT{ic}"][it]
                tr0 = b * T + ic * 512 + it * 128
                ot = opool.tile([128, C], BF16, tag="ot", name=f"ot{b}_{ic}_{it}")
                for fc in range(2):
                    pp = ps_mm.tile([128, 512], F32, tag="mm", name=f"pp{b}_{ic}_{it}_{fc}")
                    nc.tensor.matmul(
                        pp[:, :],
                        lhsT=yt[:, :],
                        rhs=wp2_sb[:, fc * 512 : fc * 512 + 512],
                        start=True,
                        stop=True,
                    )
                    nc.vector.tensor_copy(ot[:, fc * 512 : fc * 512 + 512], pp[:, :])
                nc.sync.dma_start(outp[tr0 : tr0 + 128, :], ot[:, :])

            # ---- emission: software-pipelined; qkv runs one chunk ahead of
            # the attention that consumes it. Filler pieces (qkv, proj) are
            # banked in a deadline queue and dispensed one per jt-group slot
            # so attention-heavy chunks keep a dense PE stream. ----
            work_q = []  # list of (deadline_linear_idx, fn), kept sorted

            def push_work(deadline, fn):
                work_q.append((deadline, fn))

            def flush_due(now):
                work_q.sort(key=lambda e: e[0])
                while work_q and work_q[0][0] <= now:
                    work_q.pop(0)[1]()

            slots_left = [2 * sum(ic + 1 for ic in range(NCH)) * B]  # 80

            def pop_one():
                # pop harder when the backlog outpaces the remaining slots,
                # but never more than 2 per slot (burst = serialized stall)
                n = 2 if len(work_q) > slots_left[0] else 1
                for _ in range(n):
                    if work_q:
                        work_q.pop(0)[1]()
                slots_left[0] -= 1

            alloc_qkv(0)
            emit_qkv_piece(0, 0, "x")
            load_late_consts()
            for which in ("q", "k", "v"):
                emit_qkv_piece(0, 0, which)
            for b in range(B):
                for ic in range(NCH):
                    now = b * NCH + ic
                    # next qkv chunk: (b, ic+1), rolling into (b+1, 0);
                    # must be emitted before attention (b, ic+1) -> deadline
                    nb, ncn = (b, ic + 1) if ic + 1 < NCH else (b + 1, 0)
                    if nb < B:
                        if ncn == 0:
                            alloc_qkv(nb)
                        for which in ("q", "k", "v"):
                            push_work(
                                nb * NCH + ncn,
                                lambda b_=nb, cn_=ncn, w_=which: emit_qkv_piece(b_, cn_, w_),
                            )
                    pb, pic = (b, ic - 1) if ic > 0 else (b - 1, NCH - 1)
                    if pb >= 0:
                        for it in range(4):
                            push_work(
                                now + 3,
                                lambda pb_=pb, pic_=pic, it_=it: emit_proj_piece(pb_, pic_, it_),
                            )
                    flush_due(now)
                    emit_attn_ic(b, ic, pop_one)
                if debug:
                    st = state[b]
                    nc.gpsimd.dma_start(dbg_q[:, b * T : b * T + T], st["qT"][:, :])
                    nc.gpsimd.dma_start(dbg_k[:, b * T : b * T + T], st["kT"][:, :])
                    nc.gpsimd.dma_start(
                        dbg_v[b, :, :],
                        st["va"][:, :, :, :].rearrange("p a h d -> p (a h d)"),
                    )
            while work_q:
                pop_one()
            for it in range(4):
                emit_proj_piece(B - 1, NCH - 1, it)
    if split:
        _split_waits(nc)
    return nc


# ---------------------------------------------------------------- host side


def make_tables(T):
    inv_freq = 1.0 / (ROPE_BASE ** (np.arange(0, HD, 2, dtype=np.float32) / HD))
    pos = np.arange(T, dtype=np.float32)
    freqs = pos[:, None] * inv_freq[None, :]  # [T, 32]
    cos = np.cos(freqs).astype(np.float32)  # [T, 32] (same for both halves)
    sin = np.sin(freqs).astype(np.float32)
    cosT64 = np.concatenate([cos.T, cos.T], axis=0)  # [64, T]
    sinT64 = np.concatenate([-sin.T, sin.T], axis=0)  # sign-baked rotate_half
    cosT = np.concatenate([cosT64, cosT64], axis=0).copy()  # [128, T] two heads
    sinT = np.concatenate([sinT64, sinT64], axis=0).copy()
    return cosT, sinT


def make_perm():
    # perm[k, m] = 1 iff m == (k+32) % 64 within each 64-row head block
    p = np.zeros((128, 128), dtype=np.float32)
    for hb in range(2):
        for k in range(HD):
            p[hb * HD + k, hb * HD + (k + 32) % HD] = 1.0
    return p


def make_tri():
    # tri[p, f] = 1.0 if p <= f (keep j <= i within a diagonal 128x128 tile)
    p = np.arange(128)[:, None]
    f = np.arange(128)[None, :]
    return (p <= f).astype(np.float32)


def make_in_maps(x, W_qkv, b_qkv, W_proj, n_cores):
    B, T, C = x.shape
    import ml_dtypes

    xT = np.ascontiguousarray(x.reshape(B * T, C).T.astype(ml_dtypes.bfloat16))
    cosT, sinT = make_tables(T)
    tri = make_tri()
    perm = make_perm()
    in_maps = []
    for c in range(n_cores):
        h0 = 2 * c * HD  # first head's column offset (2 heads per core)
        sl = slice(h0, h0 + 128)
        in_maps.append(
            {
                "xT": xT,
                "wq": np.ascontiguousarray(W_qkv[:, sl].astype(ml_dtypes.bfloat16)),
                "wk": np.ascontiguousarray(
                    W_qkv[:, C:][:, sl].astype(ml_dtypes.bfloat16)
                ),
                "wv": np.ascontiguousarray(
                    W_qkv[:, 2 * C :][:, sl].astype(ml_dtypes.bfloat16)
                ),
                "bq": np.ascontiguousarray(b_qkv[sl].reshape(128, 1)),
                "bk": np.ascontiguousarray(b_qkv[C:][sl].reshape(128, 1)),
                "bv": np.ascontiguousarray(b_qkv[2 * C :][sl].reshape(1, 128)),
                "wp": np.ascontiguousarray(W_proj[sl, :].astype(ml_dtypes.bfloat16)),
                "cosT": cosT.astype(ml_dtypes.bfloat16),
                "sinT": sinT.astype(ml_dtypes.bfloat16),
                "perm": perm.astype(ml_dtypes.bfloat16),
                "tri": tri.astype(ml_dtypes.bfloat16),
            }
        )
    return in_maps


_NC_CACHE = {}


def _get_nc(B, T, C):
    key = (B, T, C)
    if key not in _NC_CACHE:
        _NC_CACHE[key] = build_nc(B, T, C)
    return _NC_CACHE[key]


def kernel(x, W_qkv, b_qkv, W_proj, b_proj):
    from concourse.bass_utils import run_bass_kernel_spmd

    x = np.asarray(x, dtype=np.float32)
    W_qkv = np.asarray(W_qkv, dtype=np.float32)
    b_qkv = np.asarray(b_qkv, dtype=np.float32)
    W_proj = np.asarray(W_proj, dtype=np.float32)
    b_proj = np.asarray(b_proj, dtype=np.float32)
    B, T, C = x.shape
    n_cores = 8
    nc = _get_nc(B, T, C)
    in_maps = make_in_maps(x, W_qkv, b_qkv, W_proj, n_cores)
    res = run_bass_kernel_spmd(nc, in_maps, core_ids=list(range(n_cores)))
    out = np.zeros((B * T, C), dtype=np.float32)
    for r in res.results:
        out += r["outp"].astype(np.float32)
    out += b_proj[None, :]
    return out.reshape(B, T, C)


# revision 47
# speedup vs baseline: 1.1986x; 1.0455x over previous
"""Causal self-attention (RoPE) Trainium2 kernel, 8-way head-parallel.

Sharding: each of the 8 cores computes 2 of the 16 heads for all 4 batches
(tensor parallel over heads: W_qkv column-split, W_proj row-split). Host
pre-transposes x -> xT [C, B*T], slices per-core weights, and sum-reduces the
8 partial projection outputs (+ b_proj) — the standard row-parallel TP reduce.

Per-core dataflow (bf16 storage, bf16 matmuls, f32 PSUM):
  qT,kT = W.T @ xT   [feat, tok], bias added on PSUM evac, RoPE via
                     sign-baked cos/sin tables + rotate-half permutation
                     matmul on the PE
  v     = xT.T @ Wv  [tok, feat] directly (lhsT = x chunk), augmented with a
                     ones column per head -> v_aug [tok, (h, d|1)]
  S^T   = kT_jtile.T @ qT[i-window]   ragged on the causal diagonal
  P^T   = exp(S^T/8) (ACT, pair-tiles), triangular mask on diag 128x128 only
  y     = P^T.T @ v_aug accumulated over j tiles in PSUM ([i, d|den] layout,
                     K=128, M=128, N=65 -> 2.3x fewer PE rows than y^T form)
  y_norm= y * (1/den) per-partition (DVE), heads concatenated on free dim
  yT    via DMA XBAR transpose (off the PE), out = yT.T @ Wp per token tile
"""

import numpy as np

import concourse.bass as bass
import concourse.mybir as mybir
import concourse.tile as tile

F32 = mybir.dt.float32
BF16 = mybir.dt.bfloat16
AF = mybir.ActivationFunctionType
OP = mybir.AluOpType

# ---------------------------------------------------------------- tile patch
# This walrus build rejects >1 embedded sync-wait on sync-engine CTRL
# instructions; Tile's tail drain embeds one wait per outstanding semaphore.
# Split them across NOPs (1 wait each) before the drain.


def _patched_drain_and_barrier(self, tick_clock, wait_clock):
    from concourse.tile import ScopedClock

    nc = self.nc
    probe = nc.sync.nop(nofuse=True)
    wait_clock.add_sem_waits(probe.ins, ScopedClock({None: tick_clock.global_clock}))
    si = probe.ins.sync_info
    waits = list(si.on_wait) if si is not None and si.on_wait else []
    if len(waits) > 1:
        si.on_wait = waits[:1]
        for w in waits[1:]:
            nop = nc.sync.nop(nofuse=True)
            nsi = nop.ins.sync_info
            if nsi is None:
                nop.ins.sync_info = mybir.SyncInfo(on_wait=[w], on_update=[])
            else:
                nsi.on_wait = [w]
    nc.sync.drain()
    nc.all_engine_barrier()
    assert self.sems is not None
    popped = nc._tile_sem_poison_stack.pop()
    assert popped is self._sem_poison
    # chunk the sem clears: the range-encoded gpsimd drain (dma_reset) in this
    # walrus build rejects wide semaphore ranges ("ISA wrong length")
    sems = sorted(
        s.num if hasattr(s, "num") else s for s in self.sems.allocated().values()
    )
    for i in range(0, len(sems), 16):
        nc.clear_and_free_semaphores(sems[i : i + 16])
    nc.all_engine_barrier()


tile.TileContext._drain_and_barrier = _patched_drain_and_barrier


def _split_waits(nc):
    """Hoist all but one sync-wait per instruction onto same-engine NOPs
    (this walrus codegen supports a single embedded wait per instruction)."""
    n = 0
    for f in nc.m.functions:
        for blk in f.blocks:
            out = []
            changed = False
            for ins in blk.instructions:
                si = ins.sync_info
                if si is not None and si.on_wait and len(si.on_wait) > 1:
                    waits = list(si.on_wait)
                    for w in waits[:-1]:
                        n += 1
                        out.append(
                            mybir.InstNoOp(
                                name=f"wsplit{n}",
                                engine=ins.engine,
                                sync_info=mybir.SyncInfo(on_wait=[w], on_update=[]),
                                bass_nofuse=True,
                            )
                        )
                    si.on_wait = waits[-1:]
                    changed = True
                out.append(ins)
            if changed:
                blk.instructions = out
    return n


# ------------------------------------------------------------------- builder

HD = 64  # head dim (fixed)
ROPE_BASE = 10000.0
SCL = float(1.0 / np.sqrt(HD))


def build_nc(B, T, C, debug=False, split=True):
    """One core's program: 2 heads x B batches. T % 512 == 0, C % 128 == 0."""
    assert T % 512 == 0 and C % 128 == 0
    TOK = B * T
    KC = C // 128  # contraction chunks for QKV
    NCH = T // 512  # i-chunks per batch
    NJT = T // 128  # j-tiles per batch
    FW = 128  # qkv feature width per tensor (2 heads * 64)

    nc = bass.Bass()
    xT = nc.dram_tensor("xT", [C, TOK], BF16, kind="ExternalInput")
    wq = nc.dram_tensor("wq", [C, FW], BF16, kind="ExternalInput")
    wk = nc.dram_tensor("wk", [C, FW], BF16, kind="ExternalInput")
    wv = nc.dram_tensor("wv", [C, FW], BF16, kind="ExternalInput")
    bq = nc.dram_tensor("bq", [FW, 1], F32, kind="ExternalInput")
    bk = nc.dram_tensor("bk", [FW, 1], F32, kind="ExternalInput")
    bv = nc.dram_tensor("bv", [1, FW], F32, kind="ExternalInput")
    wp = nc.dram_tensor("wp", [FW, C], BF16, kind="ExternalInput")
    cosT = nc.dram_tensor("cosT", [FW, T], BF16, kind="ExternalInput")
    sinT = nc.dram_tensor("sinT", [FW, T], BF16, kind="ExternalInput")
    perm = nc.dram_tensor("perm", [128, 128], BF16, kind="ExternalInput")
    tri = nc.dram_tensor("tri", [128, 128], BF16, kind="ExternalInput")
    outp = nc.dram_tensor("outp", [TOK, C], BF16, kind="ExternalOutput")
    if debug:
        dbg_q = nc.dram_tensor("dbg_q", [FW, TOK], F32, kind="ExternalOutput")
        dbg_k = nc.dram_tensor("dbg_k", [FW, TOK], F32, kind="ExternalOutput")
        dbg_v = nc.dram_tensor("dbg_v", [B, 128, NJT * 2 * (HD + 1)], F32, kind="ExternalOutput")
        dbg_y = nc.dram_tensor("dbg_y", [B, 128, NJT * 128], F32, kind="ExternalOutput")
        dbg_pt = nc.dram_tensor("dbg_pt", [128, 1024], F32, kind="ExternalOutput")
        dbg_den = nc.dram_tensor("dbg_den", [128, 8], F32, kind="ExternalOutput")

    xT_r = xT[:, :].rearrange("(a p) t -> p a t", p=128)  # [128, KC, TOK]

    with tile.TileContext(nc) as tc:
        with (
            tc.tile_pool(name="const", bufs=1) as cpool,
            tc.tile_pool(name="xt", bufs=3) as xpool,
            tc.tile_pool(name="qk", bufs=2) as qkpool,
            tc.tile_pool(name="vv", bufs=2) as vpool,
            tc.tile_pool(name="rope", bufs=2) as rpool,
            tc.tile_pool(name="pt", bufs=6) as ptpool,
            tc.tile_pool(name="ysb", bufs=2) as ypool,
            tc.tile_pool(name="yt", bufs=16) as ytpool,
            tc.tile_pool(name="rcps", bufs=2) as rcpool,
            tc.tile_pool(name="outs", bufs=4) as opool,
            tc.tile_pool(name="ps_sp", bufs=2, space="PSUM") as ps_sp,
            tc.tile_pool(name="ps_y", bufs=1, space="PSUM") as ps_y,
            tc.tile_pool(name="ps_mm", bufs=2, space="PSUM") as ps_mm,
        ):
            # ---- constants ----
            # wq first, then the first x chunk (emitted by the caller right
            # after load_wq), then the rest — the SP DMA queue is in-order,
            # so this gets the first matmul started ASAP
            w_sb = {}

            def load_w(name, dram):
                t = cpool.tile([128, KC, FW], BF16, tag=name, name=name + "_sb")
                nc.sync.dma_start(
                    t[:, :, :], dram[:, :].rearrange("(a p) f -> p a f", p=128)
                )
                w_sb[name] = t

            load_w("wq", wq)
            b_sb = {}
            for name, dram in (("bq", bq), ("bk", bk)):
                t = cpool.tile([FW, 1], F32, tag=name, name=name + "_sb")
                nc.sync.dma_start(t[:, :], dram[:, :])
                b_sb[name] = t
            bv_b = cpool.tile([128, 2, HD], F32, tag="bv_b")
            nc.sync.dma_start(
                bv_b[:, :, :],
                bv[:, :].rearrange("o (h d) -> o h d", h=2).broadcast_to([128, 2, HD]),
            )
            perm_sb = cpool.tile([128, 128], BF16, tag="perm")
            nc.sync.dma_start(perm_sb[:, :], perm[:, :])
            tri_sb = cpool.tile([128, 128], BF16, tag="tri")
            nc.sync.dma_start(tri_sb[:, :], tri[:, :])
            cos_sb = cpool.tile([FW, T], BF16, tag="cos")
            sin_sb = cpool.tile([FW, T], BF16, tag="sin")
            wp2_sb = cpool.tile([FW, C], BF16, tag="wp2")

            def load_late_consts():
                load_w("wk", wk)
                load_w("wv", wv)
                nc.sync.dma_start(cos_sb[:, :], cosT[:, :])
                nc.sync.dma_start(sin_sb[:, :], sinT[:, :])
                nc.sync.dma_start(wp2_sb[:, :], wp[:, :])

            state = {}

            def alloc_qkv(b):
                st = state.setdefault(b, {})
                st["qT"] = qkpool.tile([FW, T], BF16, tag="qT", name=f"qT{b}")
                st["kT"] = qkpool.tile([FW, T], BF16, tag="kT", name=f"kT{b}")
                # v_aug[tok, jt, h, d|1]: per-head value tiles with a ones
                # column at d=HD (softmax denominator via the PV matmul)
                st["va"] = vpool.tile(
                    [128, NJT, 2, HD + 1], BF16, tag="va", name=f"va{b}"
                )
                nc.vector.memset(st["va"][:, :, :, HD], 1.0)

            def emit_qkv_piece(b, cn, which):
                """which in ('q', 'k', 'v'): one tensor's worth of a 512-token
                chunk. q/k include bias + RoPE; v is direct [tok, feat]."""
                st = state[b]
                tok0 = b * T
                ts0 = cn * 512
                if f"xt{cn}" not in st or st[f"xt{cn}"] is None:
                    # x chunk load serves q, k and v of this (b, cn)
                    xt = xpool.tile([128, KC, 512], BF16, tag="xt", name=f"xt{b}_{cn}")
                    st[f"xt{cn}"] = xt
                    nc.sync.dma_start(
                        xt[:, :, :], xT_r[:, :, tok0 + ts0 : tok0 + ts0 + 512]
                    )
                if which == "x":
                    return
                xt = st[f"xt{cn}"]
                if which in ("q", "k"):
                    wname = "wq" if which == "q" else "wk"
                    dest = st["qT"] if which == "q" else st["kT"]
                    ps = ps_mm.tile([128, 512], F32, tag="mm", name=f"qkps{b}_{cn}_{which}")
                    for kc in range(KC):
                        nc.tensor.matmul(
                            ps[:, :],
                            lhsT=w_sb[wname][:, kc, :],
                            rhs=xt[:, kc, :],
                            start=(kc == 0),
                            stop=(kc == KC - 1),
                        )
                    dch = dest[:, ts0 : ts0 + 512]
                    bias = b_sb["bq" if which == "q" else "bk"]
                    nc.vector.tensor_scalar_add(dch, ps[:, :], bias[:, :])
                    swp = ps_mm.tile([128, 512], F32, tag="mm", name=f"swp{b}_{cn}_{which}")
                    nc.tensor.matmul(
                        swp[:, :], lhsT=perm_sb[:, :], rhs=dch, start=True, stop=True
                    )
                    cc = cos_sb[:, ts0 : ts0 + 512]
                    ss = sin_sb[:, ts0 : ts0 + 512]
                    t1 = rpool.tile([128, 512], BF16, tag="t1", name=f"t1{b}_{cn}_{which}")
                    t2 = rpool.tile([128, 512], BF16, tag="t2", name=f"t2{b}_{cn}_{which}")
                    nc.gpsimd.tensor_tensor(t1[:, :], dch, cc, op=OP.mult)
                    nc.vector.tensor_tensor(t2[:, :], swp[:, :], ss, op=OP.mult)
                    nc.gpsimd.tensor_tensor(dch, t1[:, :], t2[:, :], op=OP.add)
                else:
                    # v directly in [tok, feat] layout: lhsT = x chunk
                    va = st["va"]
                    vps = ps_mm.tile([128, 4, 2, HD], F32, tag="mm", name=f"vps{b}_{cn}")
                    for s in range(4):
                        for kc in range(KC):
                            nc.tensor.matmul(
                                vps[:, s, :, :],
                                lhsT=xt[:, kc, s * 128 : s * 128 + 128],
                                rhs=w_sb["wv"][:, kc, :],
                                start=(kc == 0),
                                stop=(kc == KC - 1),
                            )
                    for s in range(4):
                        nc.vector.tensor_tensor(
                            va[:, cn * 4 + s, :, 0:HD],
                            vps[:, s, :, :],
                            bv_b[:, :, :],
                            op=OP.add,
                        )

            # ---- attention ----
            # Per (b, ic): j-tiles jt = 0..4(ic+1)-1, each S^T [128j, N] with
            # N ragged on the diagonal (i-window = [(ic+1)*512-N, (ic+1)*512)).
            # jts are processed in GROUPS of 2 (jtA, jtB): per head one
            # [128,1024] psum tile holds jtA at [0:N_A] and jtB at
            # [512:512+N_B]. The h0/h1 S matmuls for the same jt are emitted
            # adjacently: K=64 at partitions 0/64 -> disjoint PE row-groups,
            # so they run concurrently.
            def emit_s_group(b, ic, g, tag):
                st = state[b]
                qT, kT = st["qT"], st["kT"]
                jts = [2 * g, 2 * g + 1]
                Ns, i_los = [], []
                for jt in jts:
                    r = jt - 4 * ic
                    N = 512 if r < 0 else 512 - 128 * r
                    Ns.append(N)
                    i_los.append((ic + 1) * 512 - N)
                sps = {
                    h: ps_sp.tile([128, 1024], F32, tag="sp", name=f"sps{tag}_{h}")
                    for h in range(2)
                }
                for li, jt in enumerate(jts):
                    N, i_lo = Ns[li], i_los[li]
                    for h in range(2):
                        hr0 = h * HD
                        nc.tensor.matmul(
                            sps[h][:, li * 512 : li * 512 + N],
                            lhsT=kT[hr0 : hr0 + HD, jt * 128 : jt * 128 + 128],
                            rhs=qT[hr0 : hr0 + HD, i_lo : i_lo + N],
                            start=True,
                            stop=True,
                        )
                pt = {}
                for h in range(2):
                    pt[h] = ptpool.tile([128, 1024], BF16, tag="pt", name=f"pt{tag}_{h}")
                    if Ns[0] == 512:  # contiguous [0 : 512+N_B]
                        nc.scalar.activation(
                            pt[h][:, 0 : 512 + Ns[1]],
                            sps[h][:, 0 : 512 + Ns[1]],
                            AF.Exp,
                            scale=SCL,
                        )
                    else:
                        for li in range(2):
                            nc.scalar.activation(
                                pt[h][:, li * 512 : li * 512 + Ns[li]],
                                sps[h][:, li * 512 : li * 512 + Ns[li]],
                                AF.Exp,
                                scale=SCL,
                            )
                for li, jt in enumerate(jts):
                    if jt >= 4 * ic:  # triangular mask on the it==jt block
                        for h in range(2):
                            nc.gpsimd.tensor_tensor(
                                pt[h][:, li * 512 : li * 512 + 128],
                                pt[h][:, li * 512 : li * 512 + 128],
                                tri_sb[:, :],
                                op=OP.mult,
                            )
                if debug and b == 0 and ic == 0 and g == 0:
                    nc.gpsimd.dma_start(dbg_pt[:, :], pt[0][:, :])
                return (jts, Ns, i_los, pt)

            def emit_pv(b, ic, grp, yps):
                jts, Ns, i_los, pt = grp
                va = state[b]["va"]
                for li, jt in enumerate(jts):
                    N, i_lo = Ns[li], i_los[li]
                    for h in range(2):
                        for it in range(i_lo // 128, 4 * ic + 4):
                            col = li * 512 + (it * 128 - i_lo)
                            # start=True resets the has-written bits for the
                            # WHOLE psum bank: only the very first matmul into
                            # this bank may use it; later regions' first
                            # writes land on cleared bits and overwrite.
                            nc.tensor.matmul(
                                yps[h][:, it - 4 * ic, 0 : HD + 1],
                                lhsT=pt[h][:, col : col + 128],
                                rhs=va[:, jt, h, :],
                                start=(jt == 0 and it == 4 * ic),
                                stop=(jt == it),
                                skip_group_check=True,
                            )

            def emit_attn_ic(b, ic, pop_filler):
                """pop_filler(): emits one queued filler piece (or nothing).
                Called once per jt-group slot to densify the PE stream."""
                st = state[b]
                yps = {}
                for h in range(2):
                    # padded to 128 so each accumulation region is 512B-aligned
                    yps[h] = ps_y.tile(
                        [128, 4, 128], F32, tag=f"y{h}", name=f"yps{b}_{ic}_{h}"
                    )
                ng = 2 * (ic + 1)
                pend = []  # PV deferred 1 group (2 jts) behind S
                for g in range(ng):
                    cur = emit_s_group(b, ic, g, f"{b}_{ic}_{g}")
                    pop_filler()
                    if len(pend) >= 1:
                        emit_pv(b, ic, pend.pop(0), yps)
                    pend.append(cur)
                for p in pend:
                    emit_pv(b, ic, p, yps)
                # normalization: y_sb[tok, it, h, d] = yps * 1/den
                y_sb = ypool.tile([128, 4, 2, HD], BF16, tag="ysb", name=f"ysb{b}_{ic}")
                st[f"ysb{ic}"] = y_sb
                rcp = rcpool.tile([128, 2, 4], F32, tag="rcp", name=f"rcp{b}_{ic}")
                for h in range(2):
                    nc.vector.reciprocal(rcp[:, h, :], yps[h][:, :, HD])
                    nc.vector.tensor_tensor(
                        y_sb[:, :, h, :],
                        yps[h][:, :, 0:HD],
                        rcp[:, h, :].broadcast_to([128, 4, HD]),
                        op=OP.mult,
                    )
                if debug and b == 0 and ic == 0:
                    dent = rcpool.tile([128, 8], F32, tag="dent", name="dent")
                    for h in range(2):
                        nc.vector.tensor_copy(
                            dent[:, h * 4 : h * 4 + 4], yps[h][:, :, HD]
                        )
                    nc.gpsimd.dma_start(dbg_den[:, :], dent[:, :])
                # transpose each [128 tok, 128 feat] i-tile via DMA XBAR
                st[f"yT{ic}"] = []
                for it in range(4):
                    yt = ytpool.tile([128, 128], BF16, tag="yt", name=f"yt{b}_{ic}_{it}")
                    nc.sync.dma_start(yt[:, :], y_sb[:, it, :, :], transpose=True)
                    st[f"yT{ic}"].append(yt)
                if debug:
                    nc.gpsimd.dma_start(
                        dbg_y[b, :, ic * 512 : ic * 512 + 512],
                        y_sb[:, :, :, :].rearrange("p a h d -> p (a h d)"),
                    )

            def emit_proj_piece(b, ic, it):
                st = state[b]
                yt = st[f"yT{ic}"][it]
                tr0 = b * T + ic * 512 + it * 128
                ot = opool.tile([128, C], BF16, tag="ot", name=f"ot{b}_{ic}_{it}")
                for fc in range(2):
                    pp = ps_mm.tile([128, 512], F32, tag="mm", name=f"pp{b}_{ic}_{it}_{fc}")
                    nc.tensor.matmul(
                        pp[:, :],
                        lhsT=yt[:, :],
                        rhs=wp2_sb[:, fc * 512 : fc * 512 + 512],
                        start=True,
                        stop=True,
                    )
                    nc.vector.tensor_copy(ot[:, fc * 512 : fc * 512 + 512], pp[:, :])
                nc.sync.dma_start(outp[tr0 : tr0 + 128, :], ot[:, :])

            # ---- emission: software-pipelined; qkv runs one chunk ahead of
            # the attention that consumes it. Filler pieces (qkv, proj) are
            # banked in a deadline queue and dispensed one per jt-group slot
            # so attention-heavy chunks keep a dense PE stream. ----
            work_q = []  # list of (deadline_linear_idx, fn), kept sorted

            def push_work(deadline, fn):
                work_q.append((deadline, fn))

            def flush_due(now):
                work_q.sort(key=lambda e: e[0])
                while work_q and work_q[0][0] <= now:
                    work_q.pop(0)[1]()

            slots_left = [2 * sum(ic + 1 for ic in range(NCH)) * B]  # 80

            def pop_one():
                if work_q:
                    work_q.pop(0)[1]()
                slots_left[0] -= 1

            alloc_qkv(0)
            emit_qkv_piece(0, 0, "x")
            load_late_consts()
            for which in ("q", "k", "v"):
                emit_qkv_piece(0, 0, which)
            for b in range(B):
                for ic in range(NCH):
                    now = b * NCH + ic
                    # next qkv chunk: (b, ic+1), rolling into (b+1, 0);
                    # must be emitted before attention (b, ic+1) -> deadline
                    nb, ncn = (b, ic + 1) if ic + 1 < NCH else (b + 1, 0)
                    if nb < B:
                        if ncn == 0:
                            alloc_qkv(nb)
                        for which in ("q", "k", "v"):
                            push_work(
                                nb * NCH + ncn,
                                lambda b_=nb, cn_=ncn, w_=which: emit_qkv_piece(b_, cn_, w_),
                            )
                    pb, pic = (b, ic - 1) if ic > 0 else (b - 1, NCH - 1)
                    if pb >= 0:
                        for it in range(4):
                            push_work(
                                now + 3,
                                lambda pb_=pb, pic_=pic, it_=it: emit_proj_piece(pb_, pic_, it_),
                            )
                    flush_due(now)
                    emit_attn_ic(b, ic, pop_one)
                if debug:
                    st = state[b]
                    nc.gpsimd.dma_start(dbg_q[:, b * T : b * T + T], st["qT"][:, :])
                    nc.gpsimd.dma_start(dbg_k[:, b * T : b * T + T], st["kT"][:, :])
                    nc.gpsimd.dma_start(
                        dbg_v[b, :, :],
                        st["va"][:, :, :, :].rearrange("p a h d -> p (a h d)"),
                    )
            while work_q:
                pop_one()
            for it in range(4):
                emit_proj_piece(B - 1, NCH - 1, it)
    if split:
        _split_waits(nc)
    return nc


# ---------------------------------------------------------------- host side


def make_tables(T):
    inv_freq = 1.0 / (ROPE_BASE ** (np.arange(0, HD, 2, dtype=np.float32) / HD))
    pos = np.arange(T, dtype=np.float32)
    freqs = pos[:, None] * inv_freq[None, :]  # [T, 32]
    cos = np.cos(freqs).astype(np.float32)  # [T, 32] (same for both halves)
    sin = np.sin(freqs).astype(np.float32)
    cosT64 = np.concatenate([cos.T, cos.T], axis=0)  # [64, T]
    sinT64 = np.concatenate([-sin.T, sin.T], axis=0)  # sign-baked rotate_half
    cosT = np.concatenate([cosT64, cosT64], axis=0).copy()  # [128, T] two heads
    sinT = np.concatenate([sinT64, sinT64], axis=0).copy()
    return cosT, sinT


def make_perm():
    # perm[k, m] = 1 iff m == (k+32) % 64 within each 64-row head block
    p = np.zeros((128, 128), dtype=np.float32)
    for hb in range(2):
        for k in range(HD):
            p[hb * HD + k, hb * HD + (k + 32) % HD] = 1.0
    return p


def make_tri():
    # tri[p, f] = 1.0 if p <= f (keep j <= i within a diagonal 128x128 tile)
    p = np.arange(128)[:, None]
    f = np.arange(128)[None, :]
    return (p <= f).astype(np.float32)


def make_in_maps(x, W_qkv, b_qkv, W_proj, n_cores):
    B, T, C = x.shape
    import ml_dtypes

    xT = np.ascontiguousarray(x.reshape(B * T, C).T.astype(ml_dtypes.bfloat16))
    cosT, sinT = make_tables(T)
    tri = make_tri()
    perm = make_perm()
    in_maps = []
    for c in range(n_cores):
        h0 = 2 * c * HD  # first head's column offset (2 heads per core)
        sl = slice(h0, h0 + 128)
        in_maps.append(
            {
                "xT": xT,
                "wq": np.ascontiguousarray(W_qkv[:, sl].astype(ml_dtypes.bfloat16)),
                "wk": np.ascontiguousarray(
                    W_qkv[:, C:][:, sl].astype(ml_dtypes.bfloat16)
                ),
                "wv": np.ascontiguousarray(
                    W_qkv[:, 2 * C :][:, sl].astype(ml_dtypes.bfloat16)
                ),
                "bq": np.ascontiguousarray(b_qkv[sl].reshape(128, 1)),
                "bk": np.ascontiguousarray(b_qkv[C:][sl].reshape(128, 1)),
                "bv": np.ascontiguousarray(b_qkv[2 * C :][sl].reshape(1, 128)),
                "wp": np.ascontiguousarray(W_proj[sl, :].astype(ml_dtypes.bfloat16)),
                "cosT": cosT.astype(ml_dtypes.bfloat16),
                "sinT": sinT.astype(ml_dtypes.bfloat16),
                "perm": perm.astype(ml_dtypes.bfloat16),
                "tri": tri.astype(ml_dtypes.bfloat16),
            }
        )
    return in_maps


_NC_CACHE = {}


def _get_nc(B, T, C):
    key = (B, T, C)
    if key not in _NC_CACHE:
        _NC_CACHE[key] = build_nc(B, T, C)
    return _NC_CACHE[key]


def kernel(x, W_qkv, b_qkv, W_proj, b_proj):
    from concourse.bass_utils import run_bass_kernel_spmd

    x = np.asarray(x, dtype=np.float32)
    W_qkv = np.asarray(W_qkv, dtype=np.float32)
    b_qkv = np.asarray(b_qkv, dtype=np.float32)
    W_proj = np.asarray(W_proj, dtype=np.float32)
    b_proj = np.asarray(b_proj, dtype=np.float32)
    B, T, C = x.shape
    n_cores = 8
    nc = _get_nc(B, T, C)
    in_maps = make_in_maps(x, W_qkv, b_qkv, W_proj, n_cores)
    res = run_bass_kernel_spmd(nc, in_maps, core_ids=list(range(n_cores)))
    out = np.zeros((B * T, C), dtype=np.float32)
    for r in res.results:
        out += r["outp"].astype(np.float32)
    out += b_proj[None, :]
    return out.reshape(B, T, C)


# revision 48
# speedup vs baseline: 1.2092x; 1.0088x over previous
"""Causal self-attention (RoPE) Trainium2 kernel, 8-way head-parallel.

Sharding: each of the 8 cores computes 2 of the 16 heads for all 4 batches
(tensor parallel over heads: W_qkv column-split, W_proj row-split). Host
pre-transposes x -> xT [C, B*T], slices per-core weights, and sum-reduces the
8 partial projection outputs (+ b_proj) — the standard row-parallel TP reduce.

Per-core dataflow (bf16 storage, bf16 matmuls, f32 PSUM):
  qT,kT = W.T @ xT   [feat, tok], bias added on PSUM evac, RoPE via
                     sign-baked cos/sin tables + rotate-half permutation
                     matmul on the PE
  v     = xT.T @ Wv  [tok, feat] directly (lhsT = x chunk), augmented with a
                     ones column per head -> v_aug [tok, (h, d|1)]
  S^T   = kT_jtile.T @ qT[i-window]   ragged on the causal diagonal
  P^T   = exp(S^T/8) (ACT, pair-tiles), triangular mask on diag 128x128 only
  y     = P^T.T @ v_aug accumulated over j tiles in PSUM ([i, d|den] layout,
                     K=128, M=128, N=65 -> 2.3x fewer PE rows than y^T form)
  y_norm= y * (1/den) per-partition (DVE), heads concatenated on free dim
  yT    via DMA XBAR transpose (off the PE), out = yT.T @ Wp per token tile
"""

import numpy as np

import concourse.bass as bass
import concourse.mybir as mybir
import concourse.tile as tile

F32 = mybir.dt.float32
BF16 = mybir.dt.bfloat16
AF = mybir.ActivationFunctionType
OP = mybir.AluOpType

# ---------------------------------------------------------------- tile patch
# This walrus build rejects >1 embedded sync-wait on sync-engine CTRL
# instructions; Tile's tail drain embeds one wait per outstanding semaphore.
# Split them across NOPs (1 wait each) before the drain.


def _patched_drain_and_barrier(self, tick_clock, wait_clock):
    from concourse.tile import ScopedClock

    nc = self.nc
    probe = nc.sync.nop(nofuse=True)
    wait_clock.add_sem_waits(probe.ins, ScopedClock({None: tick_clock.global_clock}))
    si = probe.ins.sync_info
    waits = list(si.on_wait) if si is not None and si.on_wait else []
    if len(waits) > 1:
        si.on_wait = waits[:1]
        for w in waits[1:]:
            nop = nc.sync.nop(nofuse=True)
            nsi = nop.ins.sync_info
            if nsi is None:
                nop.ins.sync_info = mybir.SyncInfo(on_wait=[w], on_update=[])
            else:
                nsi.on_wait = [w]
    nc.sync.drain()
    nc.all_engine_barrier()
    assert self.sems is not None
    popped = nc._tile_sem_poison_stack.pop()
    assert popped is self._sem_poison
    # chunk the sem clears: the range-encoded gpsimd drain (dma_reset) in this
    # walrus build rejects wide semaphore ranges ("ISA wrong length")
    sems = sorted(
        s.num if hasattr(s, "num") else s for s in self.sems.allocated().values()
    )
    for i in range(0, len(sems), 16):
        nc.clear_and_free_semaphores(sems[i : i + 16])
    nc.all_engine_barrier()


tile.TileContext._drain_and_barrier = _patched_drain_and_barrier


def _split_waits(nc):
    """Hoist all but one sync-wait per instruction onto same-engine NOPs
    (this walrus codegen supports a single embedded wait per instruction)."""
    n = 0
    for f in nc.m.functions:
        for blk in f.blocks:
            out = []
            changed = False
            for ins in blk.instructions:
                si = ins.sync_info
                if si is not None and si.on_wait and len(si.on_wait) > 1:
                    waits = list(si.on_wait)
                    for w in waits[:-1]:
                        n += 1
                        out.append(
                            mybir.InstNoOp(
                                name=f"wsplit{n}",
                                engine=ins.engine,
                                sync_info=mybir.SyncInfo(on_wait=[w], on_update=[]),
                                bass_nofuse=True,
                            )
                        )
                    si.on_wait = waits[-1:]
                    changed = True
                out.append(ins)
            if changed:
                blk.instructions = out
    return n


# ------------------------------------------------------------------- builder

HD = 64  # head dim (fixed)
ROPE_BASE = 10000.0
SCL = float(1.0 / np.sqrt(HD))


def build_nc(B, T, C, debug=False, split=True):
    """One core's program: 2 heads x B batches. T % 512 == 0, C % 128 == 0."""
    assert T % 512 == 0 and C % 128 == 0
    TOK = B * T
    KC = C // 128  # contraction chunks for QKV
    NCH = T // 512  # i-chunks per batch
    NJT = T // 128  # j-tiles per batch
    FW = 128  # qkv feature width per tensor (2 heads * 64)

    nc = bass.Bass()
    xT = nc.dram_tensor("xT", [C, TOK], BF16, kind="ExternalInput")
    wq = nc.dram_tensor("wq", [C, FW], BF16, kind="ExternalInput")
    wk = nc.dram_tensor("wk", [C, FW], BF16, kind="ExternalInput")
    wv = nc.dram_tensor("wv", [C, FW], BF16, kind="ExternalInput")
    bq = nc.dram_tensor("bq", [FW, 1], F32, kind="ExternalInput")
    bk = nc.dram_tensor("bk", [FW, 1], F32, kind="ExternalInput")
    bv = nc.dram_tensor("bv", [1, FW], F32, kind="ExternalInput")
    wp = nc.dram_tensor("wp", [FW, C], BF16, kind="ExternalInput")
    cosT = nc.dram_tensor("cosT", [FW, T], BF16, kind="ExternalInput")
    sinT = nc.dram_tensor("sinT", [FW, T], BF16, kind="ExternalInput")
    perm = nc.dram_tensor("perm", [128, 128], BF16, kind="ExternalInput")
    tri = nc.dram_tensor("tri", [128, 128], BF16, kind="ExternalInput")
    outp = nc.dram_tensor("outp", [TOK, C], BF16, kind="ExternalOutput")
    if debug:
        dbg_q = nc.dram_tensor("dbg_q", [FW, TOK], F32, kind="ExternalOutput")
        dbg_k = nc.dram_tensor("dbg_k", [FW, TOK], F32, kind="ExternalOutput")
        dbg_v = nc.dram_tensor("dbg_v", [B, 128, NJT * 2 * (HD + 1)], F32, kind="ExternalOutput")
        dbg_y = nc.dram_tensor("dbg_y", [B, 128, NJT * 128], F32, kind="ExternalOutput")
        dbg_pt = nc.dram_tensor("dbg_pt", [128, 1024], F32, kind="ExternalOutput")
        dbg_den = nc.dram_tensor("dbg_den", [128, 8], F32, kind="ExternalOutput")

    xT_r = xT[:, :].rearrange("(a p) t -> p a t", p=128)  # [128, KC, TOK]

    with tile.TileContext(nc) as tc:
        with (
            tc.tile_pool(name="const", bufs=1) as cpool,
            tc.tile_pool(name="xt", bufs=3) as xpool,
            tc.tile_pool(name="qk", bufs=2) as qkpool,
            tc.tile_pool(name="vv", bufs=2) as vpool,
            tc.tile_pool(name="rope", bufs=2) as rpool,
            tc.tile_pool(name="pt", bufs=6) as ptpool,
            tc.tile_pool(name="ysb", bufs=2) as ypool,
            tc.tile_pool(name="yt", bufs=16) as ytpool,
            tc.tile_pool(name="rcps", bufs=2) as rcpool,
            tc.tile_pool(name="outs", bufs=4) as opool,
            tc.tile_pool(name="ps_sp", bufs=2, space="PSUM") as ps_sp,
            tc.tile_pool(name="ps_y", bufs=1, space="PSUM") as ps_y,
            tc.tile_pool(name="ps_mm", bufs=2, space="PSUM") as ps_mm,
        ):
            # ---- constants ----
            # wq first, then the first x chunk (emitted by the caller right
            # after load_wq), then the rest — the SP DMA queue is in-order,
            # so this gets the first matmul started ASAP
            w_sb = {}

            def load_w(name, dram):
                t = cpool.tile([128, KC, FW], BF16, tag=name, name=name + "_sb")
                nc.sync.dma_start(
                    t[:, :, :], dram[:, :].rearrange("(a p) f -> p a f", p=128)
                )
                w_sb[name] = t

            load_w("wq", wq)
            b_sb = {}
            for name, dram in (("bq", bq), ("bk", bk)):
                t = cpool.tile([FW, 1], F32, tag=name, name=name + "_sb")
                nc.sync.dma_start(t[:, :], dram[:, :])
                b_sb[name] = t
            bv_b = cpool.tile([128, 2, HD], F32, tag="bv_b")
            nc.sync.dma_start(
                bv_b[:, :, :],
                bv[:, :].rearrange("o (h d) -> o h d", h=2).broadcast_to([128, 2, HD]),
            )
            perm_sb = cpool.tile([128, 128], BF16, tag="perm")
            nc.sync.dma_start(perm_sb[:, :], perm[:, :])
            tri_sb = cpool.tile([128, 128], BF16, tag="tri")
            nc.sync.dma_start(tri_sb[:, :], tri[:, :])
            cos_sb = cpool.tile([FW, T], BF16, tag="cos")
            sin_sb = cpool.tile([FW, T], BF16, tag="sin")
            wp2_sb = cpool.tile([FW, C], BF16, tag="wp2")

            def load_late_consts():
                load_w("wk", wk)
                load_w("wv", wv)
                nc.sync.dma_start(cos_sb[:, :], cosT[:, :])
                nc.sync.dma_start(sin_sb[:, :], sinT[:, :])
                nc.sync.dma_start(wp2_sb[:, :], wp[:, :])

            state = {}

            def alloc_qkv(b):
                st = state.setdefault(b, {})
                st["qT"] = qkpool.tile([FW, T], BF16, tag="qT", name=f"qT{b}")
                st["kT"] = qkpool.tile([FW, T], BF16, tag="kT", name=f"kT{b}")
                # v_aug[tok, jt, h, d|1]: per-head value tiles with a ones
                # column at d=HD (softmax denominator via the PV matmul)
                st["va"] = vpool.tile(
                    [128, NJT, 2, HD + 1], BF16, tag="va", name=f"va{b}"
                )
                nc.vector.memset(st["va"][:, :, :, HD], 1.0)

            def emit_qkv_piece(b, cn, which):
                """which in ('q', 'k', 'v'): one tensor's worth of a 512-token
                chunk. q/k include bias + RoPE; v is direct [tok, feat]."""
                st = state[b]
                tok0 = b * T
                ts0 = cn * 512
                if f"xt{cn}" not in st or st[f"xt{cn}"] is None:
                    # x chunk load serves q, k and v of this (b, cn)
                    xt = xpool.tile([128, KC, 512], BF16, tag="xt", name=f"xt{b}_{cn}")
                    st[f"xt{cn}"] = xt
                    nc.sync.dma_start(
                        xt[:, :, :], xT_r[:, :, tok0 + ts0 : tok0 + ts0 + 512]
                    )
                if which == "x":
                    return
                xt = st[f"xt{cn}"]
                if which in ("q", "k"):
                    wname = "wq" if which == "q" else "wk"
                    dest = st["qT"] if which == "q" else st["kT"]
                    ps = ps_mm.tile([128, 512], F32, tag="mm", name=f"qkps{b}_{cn}_{which}")
                    for kc in range(KC):
                        nc.tensor.matmul(
                            ps[:, :],
                            lhsT=w_sb[wname][:, kc, :],
                            rhs=xt[:, kc, :],
                            start=(kc == 0),
                            stop=(kc == KC - 1),
                        )
                    dch = dest[:, ts0 : ts0 + 512]
                    bias = b_sb["bq" if which == "q" else "bk"]
                    nc.vector.tensor_scalar_add(dch, ps[:, :], bias[:, :])
                    swp = ps_mm.tile([128, 512], F32, tag="mm", name=f"swp{b}_{cn}_{which}")
                    nc.tensor.matmul(
                        swp[:, :], lhsT=perm_sb[:, :], rhs=dch, start=True, stop=True
                    )
                    cc = cos_sb[:, ts0 : ts0 + 512]
                    ss = sin_sb[:, ts0 : ts0 + 512]
                    t1 = rpool.tile([128, 512], BF16, tag="t1", name=f"t1{b}_{cn}_{which}")
                    t2 = rpool.tile([128, 512], BF16, tag="t2", name=f"t2{b}_{cn}_{which}")
                    nc.gpsimd.tensor_tensor(t1[:, :], dch, cc, op=OP.mult)
                    nc.vector.tensor_tensor(t2[:, :], swp[:, :], ss, op=OP.mult)
                    nc.gpsimd.tensor_tensor(dch, t1[:, :], t2[:, :], op=OP.add)
                else:
                    # v directly in [tok, feat] layout: lhsT = x chunk
                    va = st["va"]
                    vps = ps_mm.tile([128, 4, 2, HD], F32, tag="mm", name=f"vps{b}_{cn}")
                    for s in range(4):
                        for kc in range(KC):
                            nc.tensor.matmul(
                                vps[:, s, :, :],
                                lhsT=xt[:, kc, s * 128 : s * 128 + 128],
                                rhs=w_sb["wv"][:, kc, :],
                                start=(kc == 0),
                                stop=(kc == KC - 1),
                            )
                    for s in range(4):
                        nc.vector.tensor_tensor(
                            va[:, cn * 4 + s, :, 0:HD],
                            vps[:, s, :, :],
                            bv_b[:, :, :],
                            op=OP.add,
                        )

            # ---- attention ----
            # Per (b, ic): j-tiles jt = 0..4(ic+1)-1, each S^T [128j, N] with
            # N ragged on the diagonal (i-window = [(ic+1)*512-N, (ic+1)*512)).
            # jts are processed in GROUPS of 2 (jtA, jtB): per head one
            # [128,1024] psum tile holds jtA at [0:N_A] and jtB at
            # [512:512+N_B]. The h0/h1 S matmuls for the same jt are emitted
            # adjacently: K=64 at partitions 0/64 -> disjoint PE row-groups,
            # so they run concurrently.
            def emit_s_group(b, ic, g, tag):
                st = state[b]
                qT, kT = st["qT"], st["kT"]
                jts = [2 * g, 2 * g + 1]
                Ns, i_los = [], []
                for jt in jts:
                    r = jt - 4 * ic
                    N = 512 if r < 0 else 512 - 128 * r
                    Ns.append(N)
                    i_los.append((ic + 1) * 512 - N)
                sps = {
                    h: ps_sp.tile([128, 1024], F32, tag="sp", name=f"sps{tag}_{h}")
                    for h in range(2)
                }
                for li, jt in enumerate(jts):
                    N, i_lo = Ns[li], i_los[li]
                    for h in range(2):
                        hr0 = h * HD
                        nc.tensor.matmul(
                            sps[h][:, li * 512 : li * 512 + N],
                            lhsT=kT[hr0 : hr0 + HD, jt * 128 : jt * 128 + 128],
                            rhs=qT[hr0 : hr0 + HD, i_lo : i_lo + N],
                            start=True,
                            stop=True,
                        )
                pt = {}
                for h in range(2):
                    pt[h] = ptpool.tile([128, 1024], BF16, tag="pt", name=f"pt{tag}_{h}")
                    if Ns[0] == 512:  # contiguous [0 : 512+N_B]
                        nc.scalar.activation(
                            pt[h][:, 0 : 512 + Ns[1]],
                            sps[h][:, 0 : 512 + Ns[1]],
                            AF.Exp,
                            scale=SCL,
                        )
                    else:
                        for li in range(2):
                            nc.scalar.activation(
                                pt[h][:, li * 512 : li * 512 + Ns[li]],
                                sps[h][:, li * 512 : li * 512 + Ns[li]],
                                AF.Exp,
                                scale=SCL,
                            )
                for li, jt in enumerate(jts):
                    if jt >= 4 * ic:  # triangular mask on the it==jt block
                        for h in range(2):
                            nc.vector.tensor_tensor(
                                pt[h][:, li * 512 : li * 512 + 128],
                                pt[h][:, li * 512 : li * 512 + 128],
                                tri_sb[:, :],
                                op=OP.mult,
                            )
                if debug and b == 0 and ic == 0 and g == 0:
                    nc.gpsimd.dma_start(dbg_pt[:, :], pt[0][:, :])
                return (jts, Ns, i_los, pt)

            def emit_pv(b, ic, grp, yps):
                jts, Ns, i_los, pt = grp
                va = state[b]["va"]
                for li, jt in enumerate(jts):
                    N, i_lo = Ns[li], i_los[li]
                    for h in range(2):
                        for it in range(i_lo // 128, 4 * ic + 4):
                            col = li * 512 + (it * 128 - i_lo)
                            # start=True resets the has-written bits for the
                            # WHOLE psum bank: only the very first matmul into
                            # this bank may use it; later regions' first
                            # writes land on cleared bits and overwrite.
                            nc.tensor.matmul(
                                yps[h][:, it - 4 * ic, 0 : HD + 1],
                                lhsT=pt[h][:, col : col + 128],
                                rhs=va[:, jt, h, :],
                                start=(jt == 0 and it == 4 * ic),
                                stop=(jt == it),
                                skip_group_check=True,
                            )

            def emit_attn_ic(b, ic, pop_filler):
                """pop_filler(): emits one queued filler piece (or nothing).
                Called once per jt-group slot to densify the PE stream."""
                st = state[b]
                yps = {}
                for h in range(2):
                    # padded to 128 so each accumulation region is 512B-aligned
                    yps[h] = ps_y.tile(
                        [128, 4, 128], F32, tag=f"y{h}", name=f"yps{b}_{ic}_{h}"
                    )
                ng = 2 * (ic + 1)
                pend = []  # PV deferred 1 group (2 jts) behind S
                for g in range(ng):
                    cur = emit_s_group(b, ic, g, f"{b}_{ic}_{g}")
                    pop_filler()
                    if len(pend) >= 1:
                        emit_pv(b, ic, pend.pop(0), yps)
                    pend.append(cur)
                for p in pend:
                    emit_pv(b, ic, p, yps)
                # normalization: y_sb[tok, it, h, d] = yps * 1/den
                y_sb = ypool.tile([128, 4, 2, HD], BF16, tag="ysb", name=f"ysb{b}_{ic}")
                st[f"ysb{ic}"] = y_sb
                rcp = rcpool.tile([128, 2, 4], F32, tag="rcp", name=f"rcp{b}_{ic}")
                for h in range(2):
                    nc.vector.reciprocal(rcp[:, h, :], yps[h][:, :, HD])
                    nc.vector.tensor_tensor(
                        y_sb[:, :, h, :],
                        yps[h][:, :, 0:HD],
                        rcp[:, h, :].broadcast_to([128, 4, HD]),
                        op=OP.mult,
                    )
                if debug and b == 0 and ic == 0:
                    dent = rcpool.tile([128, 8], F32, tag="dent", name="dent")
                    for h in range(2):
                        nc.vector.tensor_copy(
                            dent[:, h * 4 : h * 4 + 4], yps[h][:, :, HD]
                        )
                    nc.gpsimd.dma_start(dbg_den[:, :], dent[:, :])
                # transpose each [128 tok, 128 feat] i-tile via DMA XBAR
                st[f"yT{ic}"] = []
                for it in range(4):
                    yt = ytpool.tile([128, 128], BF16, tag="yt", name=f"yt{b}_{ic}_{it}")
                    nc.sync.dma_start(yt[:, :], y_sb[:, it, :, :], transpose=True)
                    st[f"yT{ic}"].append(yt)
                if debug:
                    nc.gpsimd.dma_start(
                        dbg_y[b, :, ic * 512 : ic * 512 + 512],
                        y_sb[:, :, :, :].rearrange("p a h d -> p (a h d)"),
                    )

            def emit_proj_piece(b, ic, it):
                st = state[b]
                yt = st[f"yT{ic}"][it]
                tr0 = b * T + ic * 512 + it * 128
                ot = opool.tile([128, C], BF16, tag="ot", name=f"ot{b}_{ic}_{it}")
                for fc in range(2):
                    pp = ps_mm.tile([128, 512], F32, tag="mm", name=f"pp{b}_{ic}_{it}_{fc}")
                    nc.tensor.matmul(
                        pp[:, :],
                        lhsT=yt[:, :],
                        rhs=wp2_sb[:, fc * 512 : fc * 512 + 512],
                        start=True,
                        stop=True,
                    )
                    nc.vector.tensor_copy(ot[:, fc * 512 : fc * 512 + 512], pp[:, :])
                nc.sync.dma_start(outp[tr0 : tr0 + 128, :], ot[:, :])

            # ---- emission: software-pipelined; qkv runs one chunk ahead of
            # the attention that consumes it. Filler pieces (qkv, proj) are
            # banked in a deadline queue and dispensed one per jt-group slot
            # so attention-heavy chunks keep a dense PE stream. ----
            work_q = []  # list of (deadline_linear_idx, fn), kept sorted

            def push_work(deadline, fn):
                work_q.append((deadline, fn))

            def flush_due(now):
                work_q.sort(key=lambda e: e[0])
                while work_q and work_q[0][0] <= now:
                    work_q.pop(0)[1]()

            slots_left = [2 * sum(ic + 1 for ic in range(NCH)) * B]  # 80

            def pop_one():
                if work_q:
                    work_q.pop(0)[1]()
                slots_left[0] -= 1

            alloc_qkv(0)
            emit_qkv_piece(0, 0, "x")
            load_late_consts()
            for which in ("q", "k", "v"):
                emit_qkv_piece(0, 0, which)
            for b in range(B):
                for ic in range(NCH):
                    now = b * NCH + ic
                    # next qkv chunk: (b, ic+1), rolling into (b+1, 0);
                    # must be emitted before attention (b, ic+1) -> deadline
                    nb, ncn = (b, ic + 1) if ic + 1 < NCH else (b + 1, 0)
                    if nb < B:
                        if ncn == 0:
                            alloc_qkv(nb)
                        for which in ("q", "k", "v"):
                            push_work(
                                nb * NCH + ncn,
                                lambda b_=nb, cn_=ncn, w_=which: emit_qkv_piece(b_, cn_, w_),
                            )
                    pb, pic = (b, ic - 1) if ic > 0 else (b - 1, NCH - 1)
                    if pb >= 0:
                        for it in range(4):
                            push_work(
                                now + 3,
                                lambda pb_=pb, pic_=pic, it_=it: emit_proj_piece(pb_, pic_, it_),
                            )
                    flush_due(now)
                    emit_attn_ic(b, ic, pop_one)
                if debug:
                    st = state[b]
                    nc.gpsimd.dma_start(dbg_q[:, b * T : b * T + T], st["qT"][:, :])
                    nc.gpsimd.dma_start(dbg_k[:, b * T : b * T + T], st["kT"][:, :])
                    nc.gpsimd.dma_start(
                        dbg_v[b, :, :],
                        st["va"][:, :, :, :].rearrange("p a h d -> p (a h d)"),
                    )
            while work_q:
                pop_one()
            for it in range(4):
                emit_proj_piece(B - 1, NCH - 1, it)
    if split:
        _split_waits(nc)
    return nc


# ---------------------------------------------------------------- host side


def make_tables(T):
    inv_freq = 1.0 / (ROPE_BASE ** (np.arange(0, HD, 2, dtype=np.float32) / HD))
    pos = np.arange(T, dtype=np.float32)
    freqs = pos[:, None] * inv_freq[None, :]  # [T, 32]
    cos = np.cos(freqs).astype(np.float32)  # [T, 32] (same for both halves)
    sin = np.sin(freqs).astype(np.float32)
    cosT64 = np.concatenate([cos.T, cos.T], axis=0)  # [64, T]
    sinT64 = np.concatenate([-sin.T, sin.T], axis=0)  # sign-baked rotate_half
    cosT = np.concatenate([cosT64, cosT64], axis=0).copy()  # [128, T] two heads
    sinT = np.concatenate([sinT64, sinT64], axis=0).copy()
    return cosT, sinT


def make_perm():
    # perm[k, m] = 1 iff m == (k+32) % 64 within each 64-row head block
    p = np.zeros((128, 128), dtype=np.float32)
    for hb in range(2):
        for k in range(HD):
            p[hb * HD + k, hb * HD + (k + 32) % HD] = 1.0
    return p


def make_tri():
    # tri[p, f] = 1.0 if p <= f (keep j <= i within a diagonal 128x128 tile)
    p = np.arange(128)[:, None]
    f = np.arange(128)[None, :]
    return (p <= f).astype(np.float32)


def make_in_maps(x, W_qkv, b_qkv, W_proj, n_cores):
    B, T, C = x.shape
    import ml_dtypes

    xT = np.ascontiguousarray(x.reshape(B * T, C).T.astype(ml_dtypes.bfloat16))
    cosT, sinT = make_tables(T)
    tri = make_tri()
    perm = make_perm()
    in_maps = []
    for c in range(n_cores):
        h0 = 2 * c * HD  # first head's column offset (2 heads per core)
        sl = slice(h0, h0 + 128)
        in_maps.append(
            {
                "xT": xT,
                "wq": np.ascontiguousarray(W_qkv[:, sl].astype(ml_dtypes.bfloat16)),
                "wk": np.ascontiguousarray(
                    W_qkv[:, C:][:, sl].astype(ml_dtypes.bfloat16)
                ),
                "wv": np.ascontiguousarray(
                    W_qkv[:, 2 * C :][:, sl].astype(ml_dtypes.bfloat16)
                ),
                "bq": np.ascontiguousarray(b_qkv[sl].reshape(128, 1)),
                "bk": np.ascontiguousarray(b_qkv[C:][sl].reshape(128, 1)),
                "bv": np.ascontiguousarray(b_qkv[2 * C :][sl].reshape(1, 128)),
                "wp": np.ascontiguousarray(W_proj[sl, :].astype(ml_dtypes.bfloat16)),
                "cosT": cosT.astype(ml_dtypes.bfloat16),
                "sinT": sinT.astype(ml_dtypes.bfloat16),
                "perm": perm.astype(ml_dtypes.bfloat16),
                "tri": tri.astype(ml_dtypes.bfloat16),
            }
        )
    return in_maps


_NC_CACHE = {}


def _get_nc(B, T, C):
    key = (B, T, C)
    if key not in _NC_CACHE:
        _NC_CACHE[key] = build_nc(B, T, C)
    return _NC_CACHE[key]


def kernel(x, W_qkv, b_qkv, W_proj, b_proj):
    from concourse.bass_utils import run_bass_kernel_spmd

    x = np.asarray(x, dtype=np.float32)
    W_qkv = np.asarray(W_qkv, dtype=np.float32)
    b_qkv = np.asarray(b_qkv, dtype=np.float32)
    W_proj = np.asarray(W_proj, dtype=np.float32)
    b_proj = np.asarray(b_proj, dtype=np.float32)
    B, T, C = x.shape
    n_cores = 8
    nc = _get_nc(B, T, C)
    in_maps = make_in_maps(x, W_qkv, b_qkv, W_proj, n_cores)
    res = run_bass_kernel_spmd(nc, in_maps, core_ids=list(range(n_cores)))
    out = np.zeros((B * T, C), dtype=np.float32)
    for r in res.results:
        out += r["outp"].astype(np.float32)
    out += b_proj[None, :]
    return out.reshape(B, T, C)
